# revision 21
# baseline (speedup 1.0000x reference)
"""Distributed 2-layer GCN (BangaloreGCN) on 8 Trainium2 NeuronCores.

Source-partitioned strategy (node/graph parallel per the sharding hint,
with the cross-core reduction done by ReduceScatter instead of
AllGather):

  * Nodes are packed into 424 global dest tiles x 128 lanes; lanes
    [16c, 16c+16) of every tile belong to core c, so each core owns
    6784 node slots.  A color-aware greedy pack balances, for every
    (src core, dest tile) pair, the number of incoming edges to
    <= 256, so every dest tile needs exactly ceil/128 = NCH_b chunks
    (identical across cores -> one static SPMD program).
  * GCN algebra: per layer the table s = dinv*h is computed locally
    (8x less dense work), each core gathers s[src] for the edges whose
    SOURCE it owns from its local table, and scatter-adds them into a
    full-size partial accumulator [128 lanes, 424*64] with one-hot
    selection matmuls in PSUM.  A bf16 ReduceScatter then hands every
    core the complete sums for its own 16-lane slab.
  * One-hot masks are built per chunk with tensor_scalar(is_equal)
    (iota vs the chunk's dest-lane column), which runs in the DVE 4x
    perf mode.
  * Layer 2 applies W2 *before* aggregation (A@(hW2) == (A@h)W2), so
    its table rows are 32 wide: half the gather bytes and half the
    collective payload of layer 1.
"""

import sys

sys.path.insert(0, "/opt/trn_rl_repo")

import heapq

import ml_dtypes
import numpy as np

BF16 = ml_dtypes.bfloat16

# ---- problem constants (hardcoded per contest contract) ----
N_NODES = 50000
IN_CH = 128
HID = 64
HID2 = 32
BN_EPS = 1e-5

NCORES = 8
P = 128
T_ALL = 424                # global dest tiles
SLAB = 16                  # lanes per core in each tile
LOCN = T_ALL * SLAB        # local node slots per core (6784)
LT = LOCN // P             # local col-tiles (53)
NG = 64                    # chunks per dma_gather call
PAD_LANE = 200.0
TBW = 128                  # table row stride in bf16 elems (256B)
GRP1 = 8                   # dest bins per PSUM bank, layer 1 (8*64 = 512 f32)
GRP2 = 16                  # dest bins per PSUM bank, layer 2 (16*32 = 512 f32)


# ----------------------------------------------------------------------
# host-side preparation
# ----------------------------------------------------------------------
def _pack_nodes(row, col, n):
    """Assign every node an owner slab and a global dest tile + lane.

    Returns (lane, tile) per node: owner core = lane // 16."""
    deg_in = np.bincount(col, minlength=n)
    outdeg = np.bincount(row, minlength=n)

    # owner slabs: balance out-degree (work per core), capacity LOCN
    order = np.argsort(-outdeg, kind="stable")
    heap = [(0, c) for c in range(NCORES)]
    heapq.heapify(heap)
    cnt = np.zeros(NCORES, np.int64)
    owner = np.empty(n, np.int8)
    for v in order:
        load, c = heapq.heappop(heap)
        owner[v] = c
        cnt[c] += 1
        if cnt[c] < LOCN:
            heapq.heappush(heap, (load + int(outdeg[v]), c))

    # per-node in-edge color vector (color = owner of the source node)
    cc = np.zeros((n, NCORES), np.int32)
    np.add.at(cc, (col, owner[row].astype(np.int64)), 1)

    # color-aware greedy tile packing: keep max_c E_cb small
    loads = np.zeros((T_ALL, NCORES), np.int64)
    cap = np.full((T_ALL, NCORES), SLAB, np.int16)
    tile_of = np.empty(n, np.int32)
    lane_of = np.empty(n, np.int32)
    BIG = 1 << 40
    for v in np.argsort(-deg_in, kind="stable"):
        c = int(owner[v])
        score = (loads + cc[v][None, :]).max(axis=1)
        score[cap[:, c] <= 0] = BIG
        t = int(np.argmin(score))
        tile_of[v] = t
        lane_of[v] = SLAB * c + (SLAB - cap[t, c])
        loads[t] += cc[v]
        cap[t, c] -= 1
    return lane_of, tile_of, loads


def _wrap_idx(arr):
    ni = arr.shape[0]
    blk = arr.reshape(ni // 16, 16).T.astype(np.int16)
    return np.tile(blk, (8, 1))


def host_prep(x, edge_index, W1, b1, W2, b2, fcW, fcb,
              g1, be1, rm1, rv1, g2, be2, rm2, rv2):
    n = x.shape[0]
    row = np.asarray(edge_index[0], np.int64)
    col = np.asarray(edge_index[1], np.int64)

    deg = np.bincount(col, minlength=n).astype(np.float32) + 1.0
    dinv = (1.0 / np.sqrt(deg)).astype(np.float32)

    lane, tile = _pack_nodes(row, col, n)[:2]
    owner = lane // SLAB
    a = lane % SLAB
    part = a * 8 + (tile % 8)          # SBUF partition in compute layout
    ct = tile // 8                     # SBUF col-tile in compute layout
    q = ct * P + part                  # compute index within core
    # gather tables are stored in compute order: table row == q

    # ---- per-(core, bin) chunk schedule, identical across cores ----
    e_core = owner[row]
    e_src = q[row].astype(np.int16)
    e_lane = lane[col].astype(np.float32)
    e_bin = tile[col]
    cnt_cb = np.zeros((NCORES, T_ALL), np.int64)
    np.add.at(cnt_cb, (e_core, e_bin), 1)
    ncb = np.maximum(1, -(-cnt_cb.max(axis=0) // P)).astype(np.int64)
    ncht = int(ncb.sum())
    bin_chunk_off = np.concatenate([[0], np.cumsum(ncb)])  # chunk offsets
    slot_off = bin_chunk_off * P

    # ---- per-core edge streams ----
    cores = []
    for c in range(NCORES):
        m = e_core == c
        sbin = e_bin[m]
        order = np.argsort(sbin, kind="stable")
        sbin = sbin[order]
        ssrc = e_src[m][order]
        slane = e_lane[m][order]
        starts = np.searchsorted(sbin, np.arange(T_ALL))
        rank = np.arange(len(sbin)) - starts[sbin]
        pos = slot_off[sbin] + rank
        idx_stream = np.zeros(ncht * P, np.int16)
        lane_stream = np.full(ncht * P, PAD_LANE, np.float32)
        idx_stream[pos] = ssrc
        lane_stream[pos] = slane

        calls = []
        k = 0
        while k < ncht:
            L = min(NG, ncht - k)
            calls.append(_wrap_idx(idx_stream[k * P:(k + L) * P]))
            k += L
        idx_img = np.hstack(calls)
        dest_img = lane_stream.reshape(ncht, P).T.copy()

        nodes_c = np.where(owner == c)[0]
        xs = np.zeros((LOCN, IN_CH), np.float32)
        xs[q[nodes_c]] = x[nodes_c] * dinv[nodes_c, None]
        dv = np.zeros(LOCN, np.float32)
        dv[q[nodes_c]] = dinv[nodes_c]
        dvpt = dv.reshape(LT, P).T                      # [128, LT]
        cores.append(dict(
            idx=idx_img, dest=dest_img,
            xT=np.ascontiguousarray(xs.T).astype(BF16),
            dinvimg=np.ascontiguousarray(np.repeat(dvpt, HID, axis=1)),
            d2img=np.ascontiguousarray(np.repeat(dvpt, HID2, axis=1)),
        ))

    S1c = (g1 / np.sqrt(rv1 + BN_EPS)).astype(np.float32)
    T1 = ((b1 - rm1) * S1c + be1).astype(np.float32)
    S2c = (g2 / np.sqrt(rv2 + BN_EPS)).astype(np.float32)
    T2 = ((b2 - rm2) * S2c + be2).astype(np.float32)
    consts = dict(
        w1=(W1 * S1c[None, :]).astype(BF16),
        w2=(W2 * S2c[None, :]).astype(np.float32),
        t1=np.tile(T1[None, :], (P, 1)).astype(np.float32),
        t2=np.tile(T2[None, :], (P, 1)).astype(np.float32),
        fcw=np.tile(np.asarray(fcW, np.float32).reshape(1, -1), (P, 1)).astype(np.float32),
        fcb=float(np.asarray(fcb).reshape(-1)[0]),
        ncb=ncb, ncht=ncht,
        owner=owner, part=part, ct=ct,
    )
    return cores, consts


# ----------------------------------------------------------------------
# device program
# ----------------------------------------------------------------------
def _dma_gather_raw(gp, bassmod, out_ap, in_ap, idxs_ap, num_idxs, elem_size,
                    elem_step, single_packet=False, queue_num=0):
    """bass.dma_gather allowing elem_size_bytes below 256B (row stride must
    still be a multiple of 256B)."""
    import concourse.mybir as mybir
    from concourse import ap_utils
    from concourse.bass import MemorySpace, exact_div, round_up_to_multiple

    assert idxs_ap.dtype == mybir.dt.int16
    assert in_ap.dtype == out_ap.dtype
    assert in_ap.space == MemorySpace.DRAM
    assert idxs_ap.space == MemorySpace.SBUF and out_ap.space == MemorySpace.SBUF
    assert ap_utils.ap_is_contiguous(out_ap.ap[1:])
    assert ap_utils.ap_is_contiguous(idxs_ap.ap[1:])
    assert in_ap.ap[-1][1] == out_ap.ap[-1][1] == elem_size
    assert out_ap.ap[0][1] * out_ap.ap[1][1] == round_up_to_multiple(num_idxs, 128)
    assert in_ap.ap[0][0] == elem_step
    stride_bytes_256 = exact_div(elem_step * mybir.dt.size(in_ap.dtype), 256)
    assert stride_bytes_256 < 256
    return gp.add_instruction(
        mybir.InstDMAGatherAnt(
            name=bassmod.get_next_instruction_name(),
            ins=[*gp.lower_ap_dma(in_ap, for_custom_bir_dma=True),
                 gp.lower_ap(idxs_ap),
                 gp.lower_val_access(gp.to_reg(num_idxs))],
            outs=[gp.lower_ap(out_ap)],
            transpose=False,
            num_idxs=num_idxs,
            elem_size=elem_size,
            stride_bytes_256=stride_bytes_256,
            gen_mode=0,
            single_packet=single_packet,
            queue_num=queue_num,
            sbuf_tokens_per_rank=0,
            sbuf_free_dim_per_rank=0,
            sbuf_free_dim_pad_per_rank=0,
            sbuf_byte_offset=0,
        ))


def build_bass(ncb, ncht):
    import concourse.bacc as bacc
    import concourse.bass as bassm
    import concourse.mybir as mybir
    import concourse.tile as tile
    from concourse.library_config import mlp
    from concourse.masks import make_identity

    f32 = mybir.dt.float32
    bf = mybir.dt.bfloat16
    i16 = mybir.dt.int16
    Act = mybir.ActivationFunctionType
    Alu = mybir.AluOpType

    bin_chunk_off = np.concatenate([[0], np.cumsum(ncb)])

    nc = bacc.Bacc("TRN2", target_bir_lowering=False)
    xT_d = nc.dram_tensor("xT", [P, LOCN], bf, kind="ExternalInput")
    idx_d = nc.dram_tensor("idx", [P, ncht * 8], i16, kind="ExternalInput")
    dest_d = nc.dram_tensor("dest", [P, ncht], f32, kind="ExternalInput")
    dinvimg_d = nc.dram_tensor("dinvimg", [P, LT * HID], f32, kind="ExternalInput")
    d2img_d = nc.dram_tensor("d2img", [P, LT * HID2], f32, kind="ExternalInput")
    w1_d = nc.dram_tensor("w1", [IN_CH, HID], bf, kind="ExternalInput")
    w2_d = nc.dram_tensor("w2", [HID, HID2], f32, kind="ExternalInput")
    t1_d = nc.dram_tensor("t1", [P, HID], f32, kind="ExternalInput")
    t2_d = nc.dram_tensor("t2", [P, HID2], f32, kind="ExternalInput")
    fcw_d = nc.dram_tensor("fcw", [P, HID2], f32, kind="ExternalInput")
    y_d = nc.dram_tensor("y", [P, LT], f32, kind="ExternalOutput")

    with tile.TileContext(nc) as tc:
        with (
            tc.tile_pool(name="const", bufs=1) as cpool,
            tc.tile_pool(name="upart", bufs=1) as upool,
            tc.tile_pool(name="ga", bufs=3) as gapool,
            tc.tile_pool(name="sel", bufs=12) as selpool,
            tc.tile_pool(name="asb", bufs=3) as asbpool,
            tc.tile_pool(name="hts", bufs=2) as htspool,
            tc.tile_pool(name="pacc", bufs=3, space="PSUM") as pacc,
            tc.tile_pool(name="ps2", bufs=2, space="PSUM") as ps2p,
            tc.tile_pool(name="ptp", bufs=2, space="PSUM") as ptpp,
            tc.tile_pool(name="dram", bufs=1, space="DRAM") as dpool,
        ):
            nc.gpsimd.load_library(mlp)

            # ---- constants ----
            xfull = cpool.tile([P, LOCN], bf)
            nc.sync.dma_start(out=xfull[:], in_=xT_d[:])
            w1_t = cpool.tile([IN_CH, HID], bf)
            nc.sync.dma_start(out=w1_t[:], in_=w1_d[:])
            idx_t = cpool.tile([P, ncht * 8], i16)
            nc.sync.dma_start(out=idx_t[:], in_=idx_d[:])
            dest_t = cpool.tile([P, ncht], f32)
            nc.sync.dma_start(out=dest_t[:], in_=dest_d[:])
            dinvimg = cpool.tile([P, LT * HID], f32)
            nc.sync.dma_start(out=dinvimg[:], in_=dinvimg_d[:])
            d2img = cpool.tile([P, LT * HID2], f32)
            nc.sync.dma_start(out=d2img[:], in_=d2img_d[:])
            w2_t = cpool.tile([HID, HID2], f32)
            nc.sync.dma_start(out=w2_t[:], in_=w2_d[:])
            t1_t = cpool.tile([P, HID], f32)
            nc.sync.dma_start(out=t1_t[:], in_=t1_d[:])
            t2_t = cpool.tile([P, HID2], f32)
            nc.sync.dma_start(out=t2_t[:], in_=t2_d[:])
            fcw_t = cpool.tile([P, HID2], f32)
            nc.sync.dma_start(out=fcw_t[:], in_=fcw_d[:])

            iota_i = cpool.tile([P, P], mybir.dt.int32)
            nc.gpsimd.iota(iota_i[:], pattern=[[1, P]], base=0,
                           channel_multiplier=0)
            iota_b = cpool.tile([P, P], bf)
            nc.vector.tensor_copy(out=iota_b[:], in_=iota_i[:])
            ident_f = cpool.tile([P, P], f32)
            make_identity(nc, ident_f[:])

            u1f = upool.tile([P, LT, HID], f32, tag="u1")
            s2bf = upool.tile([P, LT, HID2], bf, tag="s2")
            s2ff = upool.tile([P, LT, HID2], f32, tag="s2f")
            agg1 = upool.tile([P, LT, HID], bf, tag="agg1")
            agg2 = upool.tile([P, LT, HID2], f32, tag="agg2")
            h1 = upool.tile([P, LT, HID], f32, tag="h1")
            h2 = upool.tile([P, LT, HID2], f32, tag="h2")
            s2raw = upool.tile([P, LT * HID2], f32, tag="s2raw")
            y_sb = upool.tile([P, LT], f32, tag="y")
            scr = upool.tile([P, HID2], bf, tag="scr")

            tab1 = dpool.tile([LOCN, TBW], bf)
            tab2 = dpool.tile([LOCN, TBW], bf)
            part1 = dpool.tile([P, T_ALL * HID], bf)
            part2 = dpool.tile([P, T_ALL * HID2], f32)
            rs1 = dpool.tile([SLAB, T_ALL * HID], bf)
            rs2 = dpool.tile([SLAB, T_ALL * HID2], f32)

            # ---- L1 dense: u1 = (dinv*x) @ W1' ----
            for g in range(0, LT, 8):
                gl = min(8, LT - g)
                pm = ps2p.tile([P, GRP2, HID2], f32, space="PSUM", tag="ps2")
                pmv = pm[:].rearrange("p a w -> p (a w)")
                for j in range(gl):
                    nc.tensor.matmul(out=pmv[:, j * HID:(j + 1) * HID],
                                     lhsT=xfull[:, (g + j) * P:(g + j + 1) * P],
                                     rhs=w1_t[:], start=True, stop=True)
                nc.scalar.activation(
                    out=u1f[:, g:g + gl, :].rearrange("p a w -> p (a w)"),
                    in_=pmv[:, 0:gl * HID], func=Act.Copy)
            nc.gpsimd.dma_start(
                out=bassm.AP(tensor=tab1[:].tensor, offset=0,
                             ap=[[TBW, P], [TBW * P, LT], [1, HID]]),
                in_=u1f[:])

            # ---- scatter: gather + one-hot matmul accumulate + drain ----
            def scatter(tab, width, partial, grp, acc_dt):
                tab_ap = bassm.AP(tensor=tab[:].tensor, offset=0,
                                  ap=[[TBW, LOCN], [1, width]])
                ngrp = -(-T_ALL // grp)
                acc = None
                accv = None
                call_start = 0
                call_len = 0
                ga = None
                for b in range(T_ALL):
                    gi, sl = divmod(b, grp)
                    gl = min(grp, T_ALL - gi * grp)
                    if sl == 0:
                        acc = pacc.tile([P, grp, width] if width == HID
                                        else [P, grp, width],
                                        f32, space="PSUM", tag="acc")
                        accv = acc[:].rearrange("p a w -> p (a w)")
                    for j in range(int(ncb[b])):
                        k = int(bin_chunk_off[b]) + j
                        if k == call_start + call_len:
                            call_start = k
                            call_len = min(NG, ncht - k)
                            ga = gapool.tile([P, call_len, width], bf, tag="ga")
                            _dma_gather_raw(
                                nc.gpsimd, nc, ga[:], tab_ap,
                                idx_t[:, call_start * 8:
                                      (call_start + call_len) * 8],
                                call_len * P, width, TBW)
                        sel = selpool.tile([P, P], bf, tag="sel")
                        nc.vector.tensor_scalar(
                            out=sel[:], in0=iota_b[:],
                            scalar1=dest_t[:, k:k + 1], scalar2=None,
                            op0=Alu.is_equal)
                        nc.tensor.matmul(
                            out=accv[:, sl * width:(sl + 1) * width],
                            lhsT=sel[:], rhs=ga[:, k - call_start, :],
                            start=(j == 0), stop=(j == int(ncb[b]) - 1))
                    if sl == gl - 1:
                        asb = asbpool.tile([P, grp * width], acc_dt,
                                           tag="asb")
                        nc.scalar.activation(out=asb[:, 0:gl * width],
                                             in_=accv[:, 0:gl * width],
                                             func=Act.Copy)
                        off = gi * grp * width
                        nc.sync.dma_start(
                            out=partial[:, off:off + gl * width],
                            in_=asb[:, 0:gl * width])

            scatter(tab1, HID, part1, GRP1, bf)

            nc.gpsimd.collective_compute(
                "ReduceScatter", mybir.AluOpType.add,
                replica_groups=[list(range(NCORES))],
                ins=[part1[:]], outs=[rs1[:]],
            )

            # ---- post1: h1 = relu(dinv*(agg+u1) + T1); s2 = dinv*(h1@W2') ----
            for a in range(SLAB):
                nc.sync.dma_start(
                    out=agg1[a * 8:(a + 1) * 8, :, :],
                    in_=bassm.AP(tensor=rs1[:].tensor,
                                 offset=a * T_ALL * HID,
                                 ap=[[HID, 8], [8 * HID, LT], [1, HID]]))
            u1v = u1f[:].rearrange("p c w -> p (c w)")
            h1v = h1[:].rearrange("p c w -> p (c w)")
            nc.vector.tensor_tensor(out=h1v[:],
                                    in0=agg1[:].rearrange("p c w -> p (c w)"),
                                    in1=u1v[:], op=Alu.add)
            nc.vector.tensor_tensor(out=h1v[:], in0=h1v[:], in1=dinvimg[:],
                                    op=Alu.mult)
            nc.vector.tensor_tensor(
                out=h1[:], in0=h1[:],
                in1=t1_t[:, None, :].to_broadcast([P, LT, HID]),
                op=Alu.add)
            nc.scalar.activation(out=h1v[:], in_=h1v[:], func=Act.Relu)

            # transpose h1 per 4 col-tiles (f32 psum), then apply W2
            for g in range(0, LT, 4):
                gl = min(4, LT - g)
                tp = ptpp.tile([HID, 4, P], f32, space="PSUM", tag="tp")
                for j in range(gl):
                    nc.tensor.transpose(out=tp[:, j, :],
                                        in_=h1[:, g + j, :],
                                        identity=ident_f[:])
                hts = htspool.tile([HID, 4 * P], f32, tag="hts")
                nc.scalar.activation(
                    out=hts[:, 0:gl * P],
                    in_=tp[:].rearrange("p a w -> p (a w)")[:, 0:gl * P],
                    func=Act.Copy)
                gi2, r2 = divmod(g // 4, 4)
                if r2 == 0:
                    pm2 = ps2p.tile([P, GRP2, HID2], f32, space="PSUM",
                                    tag="ps2")
                    pm2v = pm2[:].rearrange("p a w -> p (a w)")
                for j in range(gl):
                    nc.tensor.matmul(
                        out=pm2v[:, (r2 * 4 + j) * HID2:
                                 (r2 * 4 + j + 1) * HID2],
                        lhsT=hts[:, j * P:(j + 1) * P], rhs=w2_t[:],
                        start=True, stop=True)
                if r2 == 3 or g + 4 >= LT:
                    lo = gi2 * GRP2 * HID2
                    ln = (r2 * 4 + gl) * HID2
                    nc.scalar.activation(out=s2raw[:, lo:lo + ln],
                                         in_=pm2v[:, 0:ln], func=Act.Copy)
            nc.vector.tensor_tensor(
                out=s2ff[:].rearrange("p c w -> p (c w)"), in0=s2raw[:],
                in1=d2img[:], op=Alu.mult)
            nc.scalar.activation(out=s2bf[:].rearrange("p c w -> p (c w)"),
                                 in_=s2ff[:].rearrange("p c w -> p (c w)"),
                                 func=Act.Copy)
            nc.sync.dma_start(
                out=bassm.AP(tensor=tab2[:].tensor, offset=0,
                             ap=[[TBW, P], [TBW * P, LT], [1, HID2]]),
                in_=s2bf[:])

            # ---- L2 scatter ----
            scatter(tab2, HID2, part2, GRP2, f32)

            nc.gpsimd.collective_compute(
                "ReduceScatter", mybir.AluOpType.add,
                replica_groups=[list(range(NCORES))],
                ins=[part2[:]], outs=[rs2[:]],
            )

            # ---- post2 + readout ----
            for a in range(SLAB):
                nc.sync.dma_start(
                    out=agg2[a * 8:(a + 1) * 8, :, :],
                    in_=bassm.AP(tensor=rs2[:].tensor,
                                 offset=a * T_ALL * HID2,
                                 ap=[[HID2, 8], [8 * HID2, LT], [1, HID2]]))
            s2v = s2ff[:].rearrange("p c w -> p (c w)")
            h2v = h2[:].rearrange("p c w -> p (c w)")
            nc.vector.tensor_tensor(out=h2v[:],
                                    in0=agg2[:].rearrange("p c w -> p (c w)"),
                                    in1=s2v[:], op=Alu.add)
            nc.vector.tensor_tensor(out=h2v[:], in0=h2v[:], in1=d2img[:],
                                    op=Alu.mult)
            nc.vector.tensor_tensor(
                out=h2[:], in0=h2[:],
                in1=t2_t[:, None, :].to_broadcast([P, LT, HID2]),
                op=Alu.add)
            nc.scalar.activation(out=h2v[:], in_=h2v[:], func=Act.Relu)
            nc.vector.tensor_tensor(
                out=h2[:], in0=h2[:],
                in1=fcw_t[:, None, :].to_broadcast([P, LT, HID2]),
                op=Alu.mult)
            for c in range(LT):
                nc.vector.reduce_sum(out=y_sb[:, c:c + 1], in_=h2[:, c, :],
                                     axis=mybir.AxisListType.X)
            nc.sync.dma_start(out=y_d[:], in_=y_sb[:])

    nc.compile()
    return nc


# ----------------------------------------------------------------------
# entry point
# ----------------------------------------------------------------------
def prepare(inputs):
    inputs = {k: np.asarray(v) for k, v in inputs.items()}
    cores, consts = host_prep(**inputs)
    nc = build_bass(consts["ncb"], consts["ncht"])

    in_maps = []
    for c in range(NCORES):
        in_maps.append({
            "xT": cores[c]["xT"],
            "idx": cores[c]["idx"],
            "dest": cores[c]["dest"],
            "dinvimg": cores[c]["dinvimg"],
            "d2img": cores[c]["d2img"],
            "w1": consts["w1"],
            "w2": consts["w2"],
            "t1": consts["t1"],
            "t2": consts["t2"],
            "fcw": consts["fcw"],
        })
    return nc, in_maps, consts


def execute(nc, in_maps):
    from concourse.bass_utils import run_bass_kernel_spmd
    return run_bass_kernel_spmd(nc, in_maps, core_ids=list(range(NCORES)))


def unshard(res, consts):
    y = np.zeros((N_NODES, 1), np.float32)
    owner, part, ct = consts["owner"], consts["part"], consts["ct"]
    fcb = consts["fcb"]
    pc = np.stack([np.asarray(res.results[c]["y"], np.float32)
                   for c in range(NCORES)])
    y[:, 0] = pc[owner[:N_NODES], part[:N_NODES], ct[:N_NODES]] + fcb
    return y


def kernel(**inputs):
    nc, in_maps, consts = prepare(inputs)
    res = execute(nc, in_maps)
    return unshard(res, consts)


# revision 25
# speedup vs baseline: 1.1563x; 1.1563x over previous
"""Distributed 2-layer GCN (BangaloreGCN) on 8 Trainium2 NeuronCores.

Source-partitioned strategy (node/graph parallel per the sharding hint,
with the cross-core reduction done by ReduceScatter instead of
AllGather):

  * Nodes are packed into 424 global dest tiles x 128 lanes; lanes
    [16c, 16c+16) of every tile belong to core c, so each core owns
    6784 node slots.  A color-aware greedy pack balances, for every
    (src core, dest tile) pair, the number of incoming edges to
    <= 256, so every dest tile needs exactly ceil/128 = NCH_b chunks
    (identical across cores -> one static SPMD program).
  * GCN algebra: per layer the table s = dinv*h is computed locally
    (8x less dense work), each core gathers s[src] for the edges whose
    SOURCE it owns from its local table, and scatter-adds them into a
    full-size partial accumulator [128 lanes, 424*64] with one-hot
    selection matmuls in PSUM.  A bf16 ReduceScatter then hands every
    core the complete sums for its own 16-lane slab.
  * One-hot masks are built per chunk with tensor_scalar(is_equal)
    (iota vs the chunk's dest-lane column), which runs in the DVE 4x
    perf mode.
  * Layer 2 applies W2 *before* aggregation (A@(hW2) == (A@h)W2), so
    its table rows are 32 wide: half the gather bytes and half the
    collective payload of layer 1.
"""

import sys

sys.path.insert(0, "/opt/trn_rl_repo")

import heapq

import ml_dtypes
import numpy as np

BF16 = ml_dtypes.bfloat16

# ---- problem constants (hardcoded per contest contract) ----
N_NODES = 50000
IN_CH = 128
HID = 64
HID2 = 32
BN_EPS = 1e-5

NCORES = 8
P = 128
T_ALL = 424                # global dest tiles
SLAB = 16                  # lanes per core in each tile
LOCN = T_ALL * SLAB        # local node slots per core (6784)
LT = LOCN // P             # local col-tiles (53)
NG = 64                    # chunks per dma_gather call
T_A = 256                  # tiles in pipeline half A (ct 0..31); rest in B
CT_A = T_A // 8
PAD_LANE = 200.0
TBW = 128                  # table row stride in bf16 elems (256B)
GRP1 = 8                   # dest bins per PSUM bank, layer 1 (8*64 = 512 f32)
GRP2 = 16                  # dest bins per PSUM bank, layer 2 (16*32 = 512 f32)


# ----------------------------------------------------------------------
# host-side preparation
# ----------------------------------------------------------------------
def _pack_nodes(row, col, n):
    """Assign every node an owner slab and a global dest tile + lane.

    Returns (lane, tile) per node: owner core = lane // 16."""
    deg_in = np.bincount(col, minlength=n)
    outdeg = np.bincount(row, minlength=n)

    # owner slabs: balance out-degree (work per core), capacity LOCN
    order = np.argsort(-outdeg, kind="stable")
    heap = [(0, c) for c in range(NCORES)]
    heapq.heapify(heap)
    cnt = np.zeros(NCORES, np.int64)
    owner = np.empty(n, np.int8)
    for v in order:
        load, c = heapq.heappop(heap)
        owner[v] = c
        cnt[c] += 1
        if cnt[c] < LOCN:
            heapq.heappush(heap, (load + int(outdeg[v]), c))

    # per-node in-edge color vector (color = owner of the source node)
    cc = np.zeros((n, NCORES), np.int32)
    np.add.at(cc, (col, owner[row].astype(np.int64)), 1)

    # color-aware greedy tile packing: keep max_c E_cb small
    loads = np.zeros((T_ALL, NCORES), np.int64)
    cap = np.full((T_ALL, NCORES), SLAB, np.int16)
    tile_of = np.empty(n, np.int32)
    lane_of = np.empty(n, np.int32)
    BIG = 1 << 40
    for v in np.argsort(-deg_in, kind="stable"):
        c = int(owner[v])
        score = (loads + cc[v][None, :]).max(axis=1)
        score[cap[:, c] <= 0] = BIG
        t = int(np.argmin(score))
        tile_of[v] = t
        lane_of[v] = SLAB * c + (SLAB - cap[t, c])
        loads[t] += cc[v]
        cap[t, c] -= 1
    return lane_of, tile_of, loads


def _wrap_idx(arr):
    ni = arr.shape[0]
    blk = arr.reshape(ni // 16, 16).T.astype(np.int16)
    return np.tile(blk, (8, 1))


def host_prep(x, edge_index, W1, b1, W2, b2, fcW, fcb,
              g1, be1, rm1, rv1, g2, be2, rm2, rv2):
    n = x.shape[0]
    row = np.asarray(edge_index[0], np.int64)
    col = np.asarray(edge_index[1], np.int64)

    deg = np.bincount(col, minlength=n).astype(np.float32) + 1.0
    dinv = (1.0 / np.sqrt(deg)).astype(np.float32)

    lane, tile = _pack_nodes(row, col, n)[:2]
    owner = lane // SLAB
    a = lane % SLAB
    part = a * 8 + (tile % 8)          # SBUF partition in compute layout
    ct = tile // 8                     # SBUF col-tile in compute layout
    q = ct * P + part                  # compute index within core
    # gather tables are stored in compute order: table row == q

    # ---- per-(core, bin) chunk schedule, identical across cores ----
    e_core = owner[row]
    e_src = q[row].astype(np.int16)
    e_lane = lane[col].astype(np.float32)
    e_bin = tile[col]
    cnt_cb = np.zeros((NCORES, T_ALL), np.int64)
    np.add.at(cnt_cb, (e_core, e_bin), 1)
    ncb = np.maximum(1, -(-cnt_cb.max(axis=0) // P)).astype(np.int64)
    ncht = int(ncb.sum())
    bin_chunk_off = np.concatenate([[0], np.cumsum(ncb)])  # chunk offsets
    slot_off = bin_chunk_off * P

    # ---- per-core edge streams ----
    cores = []
    for c in range(NCORES):
        m = e_core == c
        sbin = e_bin[m]
        order = np.argsort(sbin, kind="stable")
        sbin = sbin[order]
        ssrc = e_src[m][order]
        slane = e_lane[m][order]
        starts = np.searchsorted(sbin, np.arange(T_ALL))
        rank = np.arange(len(sbin)) - starts[sbin]
        pos = slot_off[sbin] + rank
        idx_stream = np.zeros(ncht * P, np.int16)
        lane_stream = np.full(ncht * P, PAD_LANE, np.float32)
        idx_stream[pos] = ssrc
        lane_stream[pos] = slane

        calls = []
        half_edge = int(bin_chunk_off[T_A])
        for lo, hi in ((0, half_edge), (half_edge, ncht)):
            k = lo
            while k < hi:
                L = min(NG, hi - k)
                calls.append(_wrap_idx(idx_stream[k * P:(k + L) * P]))
                k += L
        idx_img = np.hstack(calls)
        dest_img = lane_stream.reshape(ncht, P).T.copy()

        nodes_c = np.where(owner == c)[0]
        xs = np.zeros((LOCN, IN_CH), np.float32)
        xs[q[nodes_c]] = x[nodes_c] * dinv[nodes_c, None]
        dv = np.zeros(LOCN, np.float32)
        dv[q[nodes_c]] = dinv[nodes_c]
        dvpt = dv.reshape(LT, P).T                      # [128, LT]
        cores.append(dict(
            idx=idx_img, dest=dest_img,
            xT=np.ascontiguousarray(xs.T).astype(BF16),
            dinvimg=np.ascontiguousarray(np.repeat(dvpt, HID, axis=1)),
            d2img=np.ascontiguousarray(np.repeat(dvpt, HID2, axis=1)),
        ))

    S1c = (g1 / np.sqrt(rv1 + BN_EPS)).astype(np.float32)
    T1 = ((b1 - rm1) * S1c + be1).astype(np.float32)
    S2c = (g2 / np.sqrt(rv2 + BN_EPS)).astype(np.float32)
    T2 = ((b2 - rm2) * S2c + be2).astype(np.float32)
    consts = dict(
        w1=(W1 * S1c[None, :]).astype(BF16),
        w2=(W2 * S2c[None, :]).astype(np.float32),
        t1=np.tile(T1[None, :], (P, 1)).astype(np.float32),
        t2=np.tile(T2[None, :], (P, 1)).astype(np.float32),
        fcw=np.tile(np.asarray(fcW, np.float32).reshape(1, -1), (P, 1)).astype(np.float32),
        fcb=float(np.asarray(fcb).reshape(-1)[0]),
        ncb=ncb, ncht=ncht,
        owner=owner, part=part, ct=ct,
    )
    return cores, consts


# ----------------------------------------------------------------------
# device program
# ----------------------------------------------------------------------
def _dma_gather_raw(gp, bassmod, out_ap, in_ap, idxs_ap, num_idxs, elem_size,
                    elem_step, single_packet=False, queue_num=0):
    """bass.dma_gather allowing elem_size_bytes below 256B (row stride must
    still be a multiple of 256B)."""
    import concourse.mybir as mybir
    from concourse import ap_utils
    from concourse.bass import MemorySpace, exact_div, round_up_to_multiple

    assert idxs_ap.dtype == mybir.dt.int16
    assert in_ap.dtype == out_ap.dtype
    assert in_ap.space == MemorySpace.DRAM
    assert idxs_ap.space == MemorySpace.SBUF and out_ap.space == MemorySpace.SBUF
    assert ap_utils.ap_is_contiguous(out_ap.ap[1:])
    assert ap_utils.ap_is_contiguous(idxs_ap.ap[1:])
    assert in_ap.ap[-1][1] == out_ap.ap[-1][1] == elem_size
    assert out_ap.ap[0][1] * out_ap.ap[1][1] == round_up_to_multiple(num_idxs, 128)
    assert in_ap.ap[0][0] == elem_step
    stride_bytes_256 = exact_div(elem_step * mybir.dt.size(in_ap.dtype), 256)
    assert stride_bytes_256 < 256
    return gp.add_instruction(
        mybir.InstDMAGatherAnt(
            name=bassmod.get_next_instruction_name(),
            ins=[*gp.lower_ap_dma(in_ap, for_custom_bir_dma=True),
                 gp.lower_ap(idxs_ap),
                 gp.lower_val_access(gp.to_reg(num_idxs))],
            outs=[gp.lower_ap(out_ap)],
            transpose=False,
            num_idxs=num_idxs,
            elem_size=elem_size,
            stride_bytes_256=stride_bytes_256,
            gen_mode=0,
            single_packet=single_packet,
            queue_num=queue_num,
            sbuf_tokens_per_rank=0,
            sbuf_free_dim_per_rank=0,
            sbuf_free_dim_pad_per_rank=0,
            sbuf_byte_offset=0,
        ))


def build_bass(ncb, ncht):
    import concourse.bacc as bacc
    import concourse.bass as bassm
    import concourse.mybir as mybir
    import concourse.tile as tile
    from concourse.library_config import mlp
    from concourse.masks import make_identity

    f32 = mybir.dt.float32
    bf = mybir.dt.bfloat16
    i16 = mybir.dt.int16
    Act = mybir.ActivationFunctionType
    Alu = mybir.AluOpType

    bin_chunk_off = np.concatenate([[0], np.cumsum(ncb)])
    HALVES = [(0, T_A), (T_A, T_ALL)]

    nc = bacc.Bacc("TRN2", target_bir_lowering=False)
    xT_d = nc.dram_tensor("xT", [P, LOCN], bf, kind="ExternalInput")
    idx_d = nc.dram_tensor("idx", [P, ncht * 8], i16, kind="ExternalInput")
    dest_d = nc.dram_tensor("dest", [P, ncht], f32, kind="ExternalInput")
    dinvimg_d = nc.dram_tensor("dinvimg", [P, LT * HID], f32, kind="ExternalInput")
    d2img_d = nc.dram_tensor("d2img", [P, LT * HID2], f32, kind="ExternalInput")
    w1_d = nc.dram_tensor("w1", [IN_CH, HID], bf, kind="ExternalInput")
    w2_d = nc.dram_tensor("w2", [HID, HID2], f32, kind="ExternalInput")
    t1_d = nc.dram_tensor("t1", [P, HID], f32, kind="ExternalInput")
    t2_d = nc.dram_tensor("t2", [P, HID2], f32, kind="ExternalInput")
    fcw_d = nc.dram_tensor("fcw", [P, HID2], f32, kind="ExternalInput")
    y_d = nc.dram_tensor("y", [P, LT], f32, kind="ExternalOutput")

    with tile.TileContext(nc) as tc:
        with (
            tc.tile_pool(name="const", bufs=1) as cpool,
            tc.tile_pool(name="upart", bufs=1) as upool,
            tc.tile_pool(name="ga", bufs=3) as gapool,
            tc.tile_pool(name="sel", bufs=24) as selpool,
            tc.tile_pool(name="asb", bufs=3) as asbpool,
            tc.tile_pool(name="hts", bufs=2) as htspool,
            tc.tile_pool(name="pacc", bufs=3, space="PSUM") as pacc,
            tc.tile_pool(name="ps2", bufs=2, space="PSUM") as ps2p,
            tc.tile_pool(name="ptp", bufs=2, space="PSUM") as ptpp,
            tc.tile_pool(name="dram", bufs=1, space="DRAM") as dpool,
        ):
            nc.gpsimd.load_library(mlp)

            # ---- early constants (needed by dense + scatter) ----
            xfull = cpool.tile([P, LOCN], bf)
            nc.sync.dma_start(out=xfull[:], in_=xT_d[:])
            w1_t = cpool.tile([IN_CH, HID], bf)
            nc.sync.dma_start(out=w1_t[:], in_=w1_d[:])
            idx_t = cpool.tile([P, ncht * 8], i16)
            nc.sync.dma_start(out=idx_t[:], in_=idx_d[:])
            dest_t = cpool.tile([P, ncht], f32)
            nc.sync.dma_start(out=dest_t[:], in_=dest_d[:])

            iota_i = cpool.tile([P, P], mybir.dt.int32)
            nc.gpsimd.iota(iota_i[:], pattern=[[1, P]], base=0,
                           channel_multiplier=0)
            iota_b = cpool.tile([P, P], bf)
            nc.vector.tensor_copy(out=iota_b[:], in_=iota_i[:])
            ident_f = cpool.tile([P, P], f32)
            make_identity(nc, ident_f[:])

            # late constants (not needed until post1 of half A)
            dinvimg = cpool.tile([P, LT * HID], f32)
            d2img = cpool.tile([P, LT * HID2], f32)
            w2_t = cpool.tile([HID, HID2], f32)
            t1_t = cpool.tile([P, HID], f32)
            t2_t = cpool.tile([P, HID2], f32)
            fcw_t = cpool.tile([P, HID2], f32)

            u1f = upool.tile([P, LT, HID], f32, tag="u1")
            u1bf = upool.tile([P, LT, HID], bf, tag="u1b")
            s2bf = upool.tile([P, LT, HID2], bf, tag="s2")
            s2ff = upool.tile([P, LT, HID2], f32, tag="s2f")
            agg1 = upool.tile([P, LT, HID], bf, tag="agg1")
            agg2 = upool.tile([P, LT, HID2], f32, tag="agg2")
            h1 = upool.tile([P, LT, HID], f32, tag="h1")
            h2 = upool.tile([P, LT, HID2], f32, tag="h2")
            s2raw = upool.tile([P, LT * HID2], f32, tag="s2raw")
            y_sb = upool.tile([P, LT], f32, tag="y")

            tab1 = dpool.tile([LOCN, TBW], bf)
            tab2 = dpool.tile([LOCN, TBW], bf)
            part1a = dpool.tile([P, T_A * HID], bf)
            part1b = dpool.tile([P, (T_ALL - T_A) * HID], bf)
            part2a = dpool.tile([P, T_A * HID2], f32)
            part2b = dpool.tile([P, (T_ALL - T_A) * HID2], f32)
            rs1a = dpool.tile([SLAB, T_A * HID], bf)
            rs1b = dpool.tile([SLAB, (T_ALL - T_A) * HID], bf)
            rs2a = dpool.tile([SLAB, T_A * HID2], f32)
            rs2b = dpool.tile([SLAB, (T_ALL - T_A) * HID2], f32)

            # ---- L1 dense: u1 = (dinv*x) @ W1' (f32 + bf16 copies) ----
            for g in range(0, LT, 8):
                gl = min(8, LT - g)
                pm = ps2p.tile([P, GRP2, HID2], f32, space="PSUM", tag="ps2")
                pmv = pm[:].rearrange("p a w -> p (a w)")
                for j in range(gl):
                    nc.tensor.matmul(out=pmv[:, j * HID:(j + 1) * HID],
                                     lhsT=xfull[:, (g + j) * P:(g + j + 1) * P],
                                     rhs=w1_t[:], start=True, stop=True)
                nc.scalar.activation(
                    out=u1f[:, g:g + gl, :].rearrange("p a w -> p (a w)"),
                    in_=pmv[:, 0:gl * HID], func=Act.Copy)
                nc.scalar.activation(
                    out=u1bf[:, g:g + gl, :].rearrange("p a w -> p (a w)"),
                    in_=pmv[:, 0:gl * HID], func=Act.Copy)
            nc.sync.dma_start(
                out=bassm.AP(tensor=tab1[:].tensor, offset=0,
                             ap=[[TBW, P], [TBW * P, LT], [1, HID]]),
                in_=u1bf[:])

            # ---- scatter a range of dest tiles ----
            def scatter(tab, width, partial, grp, acc_dt, t0, t1):
                tab_ap = bassm.AP(tensor=tab[:].tensor, offset=0,
                                  ap=[[TBW, LOCN], [1, width]])
                acc = None
                accv = None
                call_start = 0
                call_len = 0
                ga = None
                for b in range(t0, t1):
                    gi, sl = divmod(b - t0, grp)
                    gl = min(grp, t1 - t0 - gi * grp)
                    if sl == 0:
                        acc = pacc.tile([P, grp, width], f32, space="PSUM",
                                        tag="acc")
                        accv = acc[:].rearrange("p a w -> p (a w)")
                    for j in range(int(ncb[b])):
                        k = int(bin_chunk_off[b]) + j
                        if k >= call_start + call_len:
                            call_start = k
                            call_len = min(NG, int(bin_chunk_off[t1]) - k)
                            ga = gapool.tile([P, call_len, width], bf, tag="ga")
                            _dma_gather_raw(
                                nc.gpsimd, nc, ga[:], tab_ap,
                                idx_t[:, call_start * 8:
                                      (call_start + call_len) * 8],
                                call_len * P, width, TBW)
                        sel = selpool.tile([P, P], bf, tag="sel")
                        nc.vector.tensor_scalar(
                            out=sel[:], in0=iota_b[:],
                            scalar1=dest_t[:, k:k + 1], scalar2=None,
                            op0=Alu.is_equal)
                        nc.tensor.matmul(
                            out=accv[:, sl * width:(sl + 1) * width],
                            lhsT=sel[:], rhs=ga[:, k - call_start, :],
                            start=(j == 0), stop=(j == int(ncb[b]) - 1))
                    if sl == gl - 1:
                        asb = asbpool.tile([P, grp * width], acc_dt, tag="asb")
                        nc.scalar.activation(out=asb[:, 0:gl * width],
                                             in_=accv[:, 0:gl * width],
                                             func=Act.Copy)
                        off = gi * grp * width
                        nc.sync.dma_start(
                            out=partial[:, off:off + gl * width],
                            in_=asb[:, 0:gl * width])

            def reduce_scatter(partial, rsbuf):
                nc.gpsimd.collective_compute(
                    "ReduceScatter", mybir.AluOpType.add,
                    replica_groups=[list(range(NCORES))],
                    ins=[partial[:]],
                    outs=[rsbuf[:]],
                )

            def regroup(rsbuf, aggt, width, t0, t1):
                c0, c1 = t0 // 8, t1 // 8
                ctn = c1 - c0
                for a in range(SLAB):
                    nc.sync.dma_start(
                        out=aggt[a * 8:(a + 1) * 8, c0:c1, :],
                        in_=bassm.AP(tensor=rsbuf[:].tensor,
                                     offset=a * (t1 - t0) * width,
                                     ap=[[width, 8], [8 * width, ctn],
                                         [1, width]]))

            # post1 for a ct range: h1 = relu((agg+u1)*dinv + T1);
            # s2 = dinv*(h1 @ W2'); write tab2 rows
            def post1(c0, c1):
                sl3 = (slice(None), slice(c0, c1), slice(None))
                fl = (slice(None), slice(c0 * HID, c1 * HID))
                nc.vector.tensor_tensor(out=h1[sl3], in0=agg1[sl3],
                                        in1=u1f[sl3], op=Alu.add)
                nc.vector.tensor_tensor(
                    out=h1[sl3].rearrange("p c w -> p (c w)"),
                    in0=h1[sl3].rearrange("p c w -> p (c w)"),
                    in1=dinvimg[fl], op=Alu.mult)
                nc.vector.tensor_tensor(
                    out=h1[sl3], in0=h1[sl3],
                    in1=t1_t[:, None, :].to_broadcast([P, c1 - c0, HID]),
                    op=Alu.add)
                nc.scalar.activation(
                    out=h1[sl3].rearrange("p c w -> p (c w)"),
                    in_=h1[sl3].rearrange("p c w -> p (c w)"), func=Act.Relu)
                pm2 = None
                pm2v = None
                nmm = 0
                mm_lo = c0
                for g in range(c0, c1, 4):
                    gl = min(4, c1 - g)
                    tp = ptpp.tile([HID, 4, P], f32, space="PSUM", tag="tp")
                    for j in range(gl):
                        nc.tensor.transpose(out=tp[:, j, :],
                                            in_=h1[:, g + j, :],
                                            identity=ident_f[:])
                    hts = htspool.tile([HID, 4 * P], f32, tag="hts")
                    nc.scalar.activation(
                        out=hts[:, 0:gl * P],
                        in_=tp[:].rearrange("p a w -> p (a w)")[:, 0:gl * P],
                        func=Act.Copy)
                    if nmm == 0:
                        pm2 = ps2p.tile([P, GRP2, HID2], f32, space="PSUM",
                                        tag="ps2")
                        pm2v = pm2[:].rearrange("p a w -> p (a w)")
                        mm_lo = g
                    for j in range(gl):
                        nc.tensor.matmul(
                            out=pm2v[:, (nmm + j) * HID2:(nmm + j + 1) * HID2],
                            lhsT=hts[:, j * P:(j + 1) * P], rhs=w2_t[:],
                            start=True, stop=True)
                    nmm += gl
                    if nmm >= GRP2 or g + 4 >= c1:
                        nc.scalar.activation(
                            out=s2raw[:, mm_lo * HID2:(mm_lo + nmm) * HID2],
                            in_=pm2v[:, 0:nmm * HID2], func=Act.Copy)
                        nmm = 0
                fl2 = (slice(None), slice(c0 * HID2, c1 * HID2))
                nc.vector.tensor_tensor(
                    out=s2ff[sl3].rearrange("p c w -> p (c w)"),
                    in0=s2raw[fl2], in1=d2img[fl2], op=Alu.mult)
                nc.scalar.activation(
                    out=s2bf[sl3].rearrange("p c w -> p (c w)"),
                    in_=s2ff[sl3].rearrange("p c w -> p (c w)"), func=Act.Copy)
                nc.sync.dma_start(
                    out=bassm.AP(tensor=tab2[:].tensor, offset=c0 * P * TBW,
                                 ap=[[TBW, P], [TBW * P, c1 - c0], [1, HID2]]),
                    in_=s2bf[sl3])

            def post2(c0, c1):
                sl3 = (slice(None), slice(c0, c1), slice(None))
                fl2 = (slice(None), slice(c0 * HID2, c1 * HID2))
                nc.vector.tensor_tensor(out=h2[sl3], in0=agg2[sl3],
                                        in1=s2ff[sl3], op=Alu.add)
                nc.vector.tensor_tensor(
                    out=h2[sl3].rearrange("p c w -> p (c w)"),
                    in0=h2[sl3].rearrange("p c w -> p (c w)"),
                    in1=d2img[fl2], op=Alu.mult)
                nc.vector.tensor_tensor(
                    out=h2[sl3], in0=h2[sl3],
                    in1=t2_t[:, None, :].to_broadcast([P, c1 - c0, HID2]),
                    op=Alu.add)
                nc.scalar.activation(
                    out=h2[sl3].rearrange("p c w -> p (c w)"),
                    in_=h2[sl3].rearrange("p c w -> p (c w)"), func=Act.Relu)
                nc.vector.tensor_tensor(
                    out=h2[sl3], in0=h2[sl3],
                    in1=fcw_t[:, None, :].to_broadcast([P, c1 - c0, HID2]),
                    op=Alu.mult)
                for c in range(c0, c1):
                    nc.vector.reduce_sum(out=y_sb[:, c:c + 1],
                                         in_=h2[:, c, :],
                                         axis=mybir.AxisListType.X)

            # ---- layer 1, half-pipelined ----
            scatter(tab1, HID, part1a, GRP1, bf, 0, T_A)
            reduce_scatter(part1a, rs1a)
            # late consts load during half B scatter
            nc.sync.dma_start(out=dinvimg[:], in_=dinvimg_d[:])
            nc.sync.dma_start(out=d2img[:], in_=d2img_d[:])
            nc.sync.dma_start(out=w2_t[:], in_=w2_d[:])
            nc.sync.dma_start(out=t1_t[:], in_=t1_d[:])
            nc.sync.dma_start(out=t2_t[:], in_=t2_d[:])
            nc.sync.dma_start(out=fcw_t[:], in_=fcw_d[:])
            scatter(tab1, HID, part1b, GRP1, bf, T_A, T_ALL)
            reduce_scatter(part1b, rs1b)
            regroup(rs1a, agg1, HID, 0, T_A)
            post1(0, CT_A)
            regroup(rs1b, agg1, HID, T_A, T_ALL)
            post1(CT_A, LT)

            # ---- layer 2, half-pipelined ----
            scatter(tab2, HID2, part2a, GRP2, f32, 0, T_A)
            reduce_scatter(part2a, rs2a)
            scatter(tab2, HID2, part2b, GRP2, f32, T_A, T_ALL)
            reduce_scatter(part2b, rs2b)
            regroup(rs2a, agg2, HID2, 0, T_A)
            post2(0, CT_A)
            regroup(rs2b, agg2, HID2, T_A, T_ALL)
            post2(CT_A, LT)
            nc.sync.dma_start(out=y_d[:], in_=y_sb[:])

    nc.compile()
    return nc


# ----------------------------------------------------------------------
# entry point
# ----------------------------------------------------------------------
def prepare(inputs):
    inputs = {k: np.asarray(v) for k, v in inputs.items()}
    cores, consts = host_prep(**inputs)
    nc = build_bass(consts["ncb"], consts["ncht"])

    in_maps = []
    for c in range(NCORES):
        in_maps.append({
            "xT": cores[c]["xT"],
            "idx": cores[c]["idx"],
            "dest": cores[c]["dest"],
            "dinvimg": cores[c]["dinvimg"],
            "d2img": cores[c]["d2img"],
            "w1": consts["w1"],
            "w2": consts["w2"],
            "t1": consts["t1"],
            "t2": consts["t2"],
            "fcw": consts["fcw"],
        })
    return nc, in_maps, consts


def execute(nc, in_maps):
    from concourse.bass_utils import run_bass_kernel_spmd
    return run_bass_kernel_spmd(nc, in_maps, core_ids=list(range(NCORES)))


def unshard(res, consts):
    y = np.zeros((N_NODES, 1), np.float32)
    owner, part, ct = consts["owner"], consts["part"], consts["ct"]
    fcb = consts["fcb"]
    pc = np.stack([np.asarray(res.results[c]["y"], np.float32)
                   for c in range(NCORES)])
    y[:, 0] = pc[owner[:N_NODES], part[:N_NODES], ct[:N_NODES]] + fcb
    return y


def kernel(**inputs):
    nc, in_maps, consts = prepare(inputs)
    res = execute(nc, in_maps)
    return unshard(res, consts)


# revision 42
# speedup vs baseline: 1.5072x; 1.3035x over previous
"""Distributed 2-layer GCN (BangaloreGCN) on 8 Trainium2 NeuronCores.

Source-partitioned strategy (node/graph parallel per the sharding hint,
with the cross-core reduction done by ReduceScatter instead of
AllGather):

  * Nodes are packed into 424 global dest tiles x 128 lanes; lanes
    [16c, 16c+16) of every tile belong to core c, so each core owns
    6784 node slots.  A color-aware greedy pack balances, for every
    (src core, dest tile) pair, the number of incoming edges to
    <= 256, so every dest tile needs exactly ceil/128 = NCH_b chunks
    (identical across cores -> one static SPMD program).
  * GCN algebra: per layer the table s = dinv*h is computed locally
    (8x less dense work), each core gathers s[src] for the edges whose
    SOURCE it owns from its local table, and scatter-adds them into a
    full-size partial accumulator [128 lanes, 424*64] with one-hot
    selection matmuls in PSUM.  A bf16 ReduceScatter then hands every
    core the complete sums for its own 16-lane slab.
  * One-hot masks are built per chunk with tensor_scalar(is_equal)
    (iota vs the chunk's dest-lane column), which runs in the DVE 4x
    perf mode.
  * Layer 2 applies W2 *before* aggregation (A@(hW2) == (A@h)W2), so
    its table rows are 32 wide: half the gather bytes and half the
    collective payload of layer 1.
"""

import sys

sys.path.insert(0, "/opt/trn_rl_repo")

import heapq

import ml_dtypes
import numpy as np

BF16 = ml_dtypes.bfloat16

# ---- problem constants (hardcoded per contest contract) ----
N_NODES = 50000
IN_CH = 128
HID = 64
HID2 = 32
BN_EPS = 1e-5

NCORES = 8
P = 128
T_ALL = 424                # global dest tiles
SLAB = 16                  # lanes per core in each tile
LOCN = T_ALL * SLAB        # local node slots per core (6784)
LT = LOCN // P             # local col-tiles (53)
NG = 64                    # chunks per dma_gather call
T_A = 256                  # tiles in pipeline half A (ct 0..31); rest in B
CT_A = T_A // 8
PAD_LANE = 200.0
TBW = 128                  # table row stride in bf16 elems (256B)
GRP1 = 8                   # dest bins per PSUM bank, layer 1 (8*64 = 512 f32)
GRP2 = 16                  # dest bins per PSUM bank, layer 2 (16*32 = 512 f32)


# ----------------------------------------------------------------------
# host-side preparation
# ----------------------------------------------------------------------
def _pack_nodes(row, col, n):
    """Assign every node an owner slab and a global dest tile + lane.

    Returns (lane, tile) per node: owner core = lane // 16."""
    deg_in = np.bincount(col, minlength=n)
    outdeg = np.bincount(row, minlength=n)

    # owner slabs: balance out-degree (work per core), capacity LOCN
    order = np.argsort(-outdeg, kind="stable")
    heap = [(0, c) for c in range(NCORES)]
    heapq.heapify(heap)
    cnt = np.zeros(NCORES, np.int64)
    owner = np.empty(n, np.int8)
    for v in order:
        load, c = heapq.heappop(heap)
        owner[v] = c
        cnt[c] += 1
        if cnt[c] < LOCN:
            heapq.heappush(heap, (load + int(outdeg[v]), c))

    # per-node in-edge color vector (color = owner of the source node)
    cc = np.zeros((n, NCORES), np.int32)
    np.add.at(cc, (col, owner[row].astype(np.int64)), 1)

    # color-aware greedy tile packing: keep max_c E_cb under the per-tile
    # chunk budget (light tiles aim for 1 chunk, the rest for 2)
    loads = np.zeros((T_ALL, NCORES), np.int64)
    cap = np.full((T_ALL, NCORES), SLAB, np.int16)
    budget = np.full(T_ALL, 2 * P, np.float64)
    budget[T_ALL - NLIGHT:] = P
    tile_of = np.empty(n, np.int32)
    lane_of = np.empty(n, np.int32)
    BIG = 1 << 40
    for v in np.argsort(-deg_in, kind="stable"):
        c = int(owner[v])
        score = (loads + cc[v][None, :]).max(axis=1) / budget
        score[cap[:, c] <= 0] = BIG
        t = int(np.argmin(score))
        tile_of[v] = t
        lane_of[v] = SLAB * c + (SLAB - cap[t, c])
        loads[t] += cc[v]
        cap[t, c] -= 1
    return lane_of, tile_of, loads


def _wrap_idx(arr):
    ni = arr.shape[0]
    blk = arr.reshape(ni // 16, 16).T.astype(np.int16)
    return np.tile(blk, (8, 1))


def host_prep(x, edge_index, W1, b1, W2, b2, fcW, fcb,
              g1, be1, rm1, rv1, g2, be2, rm2, rv2):
    n = x.shape[0]
    row = np.asarray(edge_index[0], np.int64)
    col = np.asarray(edge_index[1], np.int64)

    deg = np.bincount(col, minlength=n).astype(np.float32) + 1.0
    dinv = (1.0 / np.sqrt(deg)).astype(np.float32)

    lane, tile = _pack_nodes(row, col, n)[:2]
    owner = lane // SLAB
    a = lane % SLAB
    part = a * 8 + (tile % 8)          # SBUF partition in compute layout
    ct = tile // 8                     # SBUF col-tile in compute layout
    q = ct * P + part                  # compute index within core
    # gather tables are stored in compute order: table row == q

    # ---- per-(core, bin) chunk schedule, identical across cores ----
    e_core = owner[row]
    e_src = q[row].astype(np.int16)
    e_lane = lane[col].astype(np.float32)
    e_bin = tile[col]
    cnt_cb = np.zeros((NCORES, T_ALL), np.int64)
    np.add.at(cnt_cb, (e_core, e_bin), 1)
    ncb = np.maximum(1, -(-cnt_cb.max(axis=0) // P)).astype(np.int64)
    ncht = int(ncb.sum())
    bin_chunk_off = np.concatenate([[0], np.cumsum(ncb)])  # chunk offsets
    slot_off = bin_chunk_off * P

    # ---- per-core edge streams ----
    cores = []
    for c in range(NCORES):
        m = e_core == c
        sbin = e_bin[m]
        order = np.argsort(sbin, kind="stable")
        sbin = sbin[order]
        ssrc = e_src[m][order]
        slane = e_lane[m][order]
        starts = np.searchsorted(sbin, np.arange(T_ALL))
        rank = np.arange(len(sbin)) - starts[sbin]
        pos = slot_off[sbin] + rank
        idx_stream = np.zeros(ncht * P, np.int16)
        lane_stream = np.full(ncht * P, PAD_LANE, np.float32)
        idx_stream[pos] = ssrc
        lane_stream[pos] = slane

        calls = []
        half_edge = int(bin_chunk_off[T_A])
        for lo, hi in ((0, half_edge), (half_edge, ncht)):
            k = lo
            while k < hi:
                L = min(NG, hi - k)
                calls.append(_wrap_idx(idx_stream[k * P:(k + L) * P]))
                k += L
        idx_img = np.hstack(calls)
        dest_img = lane_stream.reshape(ncht, P).T.copy()

        nodes_c = np.where(owner == c)[0]
        xs = np.zeros((LOCN, IN_CH), np.float32)
        xs[q[nodes_c]] = x[nodes_c] * dinv[nodes_c, None]
        dv = np.zeros(LOCN, np.float32)
        dv[q[nodes_c]] = dinv[nodes_c]
        dvpt = dv.reshape(LT, P).T                      # [128, LT]
        cores.append(dict(
            idx=idx_img, dest=dest_img,
            xT=np.ascontiguousarray(xs.T).astype(BF16),
            dinvimg=np.ascontiguousarray(np.repeat(dvpt, HID, axis=1)).astype(BF16),
            d2img=np.ascontiguousarray(np.repeat(dvpt, HID2, axis=1)).astype(BF16),
        ))

    S1c = (g1 / np.sqrt(rv1 + BN_EPS)).astype(np.float32)
    T1 = ((b1 - rm1) * S1c + be1).astype(np.float32)
    S2c = (g2 / np.sqrt(rv2 + BN_EPS)).astype(np.float32)
    T2 = ((b2 - rm2) * S2c + be2).astype(np.float32)
    consts = dict(
        w1=(W1 * S1c[None, :]).astype(BF16),
        w2=(W2 * S2c[None, :]).astype(np.float32),
        t1=np.tile(T1[None, :], (P, 1)).astype(np.float32),
        t2=np.tile(T2[None, :], (P, 1)).astype(np.float32),
        fcw=np.tile(np.asarray(fcW, np.float32).reshape(1, -1), (P, 1)).astype(np.float32),
        fcb=float(np.asarray(fcb).reshape(-1)[0]),
        ncb=ncb, ncht=ncht,
        owner=owner, part=part, ct=ct,
    )
    return cores, consts


# ----------------------------------------------------------------------
# device program
# ----------------------------------------------------------------------
def _dma_gather_raw(gp, bassmod, out_ap, in_ap, idxs_ap, num_idxs, elem_size,
                    elem_step, single_packet=False, queue_num=0):
    """bass.dma_gather allowing elem_size_bytes below 256B (row stride must
    still be a multiple of 256B)."""
    import concourse.mybir as mybir
    from concourse import ap_utils
    from concourse.bass import MemorySpace, exact_div, round_up_to_multiple

    assert idxs_ap.dtype == mybir.dt.int16
    assert in_ap.dtype == out_ap.dtype
    assert in_ap.space == MemorySpace.DRAM
    assert idxs_ap.space == MemorySpace.SBUF and out_ap.space == MemorySpace.SBUF
    assert ap_utils.ap_is_contiguous(out_ap.ap[1:])
    assert ap_utils.ap_is_contiguous(idxs_ap.ap[1:])
    assert in_ap.ap[-1][1] == out_ap.ap[-1][1] == elem_size
    assert out_ap.ap[0][1] * out_ap.ap[1][1] == round_up_to_multiple(num_idxs, 128)
    assert in_ap.ap[0][0] == elem_step
    stride_bytes_256 = exact_div(elem_step * mybir.dt.size(in_ap.dtype), 256)
    assert stride_bytes_256 < 256
    return gp.add_instruction(
        mybir.InstDMAGatherAnt(
            name=bassmod.get_next_instruction_name(),
            ins=[*gp.lower_ap_dma(in_ap, for_custom_bir_dma=True),
                 gp.lower_ap(idxs_ap),
                 gp.lower_val_access(gp.to_reg(num_idxs))],
            outs=[gp.lower_ap(out_ap)],
            transpose=False,
            num_idxs=num_idxs,
            elem_size=elem_size,
            stride_bytes_256=stride_bytes_256,
            gen_mode=0,
            single_packet=single_packet,
            queue_num=queue_num,
            sbuf_tokens_per_rank=0,
            sbuf_free_dim_per_rank=0,
            sbuf_free_dim_pad_per_rank=0,
            sbuf_byte_offset=0,
        ))


def build_bass(ncb, ncht):
    import concourse.bacc as bacc
    import concourse.bass as bassm
    import concourse.mybir as mybir
    import concourse.tile as tile
    from concourse.library_config import mlp
    from concourse.masks import make_identity

    f32 = mybir.dt.float32
    bf = mybir.dt.bfloat16
    i16 = mybir.dt.int16
    Act = mybir.ActivationFunctionType
    Alu = mybir.AluOpType

    bin_chunk_off = np.concatenate([[0], np.cumsum(ncb)])
    HALVES = [(0, T_A), (T_A, T_ALL)]

    nc = bacc.Bacc("TRN2", target_bir_lowering=False)
    xT_d = nc.dram_tensor("xT", [P, LOCN], bf, kind="ExternalInput")
    idx_d = nc.dram_tensor("idx", [P, ncht * 8], i16, kind="ExternalInput")
    dest_d = nc.dram_tensor("dest", [P, ncht], f32, kind="ExternalInput")
    dinvimg_d = nc.dram_tensor("dinvimg", [P, LT * HID], bf, kind="ExternalInput")
    d2img_d = nc.dram_tensor("d2img", [P, LT * HID2], bf, kind="ExternalInput")
    w1_d = nc.dram_tensor("w1", [IN_CH, HID], bf, kind="ExternalInput")
    w2_d = nc.dram_tensor("w2", [HID, HID2], f32, kind="ExternalInput")
    t1_d = nc.dram_tensor("t1", [P, HID], f32, kind="ExternalInput")
    t2_d = nc.dram_tensor("t2", [P, HID2], f32, kind="ExternalInput")
    fcw_d = nc.dram_tensor("fcw", [P, HID2], f32, kind="ExternalInput")
    y_d = nc.dram_tensor("y", [P, LT], f32, kind="ExternalOutput")

    with tile.TileContext(nc) as tc:
        with (
            tc.tile_pool(name="const", bufs=1) as cpool,
            tc.tile_pool(name="upart", bufs=1) as upool,
            tc.tile_pool(name="ga", bufs=3) as gapool,
            tc.tile_pool(name="sel", bufs=24) as selpool,
            tc.tile_pool(name="asb", bufs=3) as asbpool,
            tc.tile_pool(name="hts", bufs=2) as htspool,
            tc.tile_pool(name="pacc", bufs=3, space="PSUM") as pacc,
            tc.tile_pool(name="ps2", bufs=2, space="PSUM") as ps2p,
            tc.tile_pool(name="ptp", bufs=2, space="PSUM") as ptpp,
            tc.tile_pool(name="dram", bufs=1, space="DRAM") as dpool,
        ):
            nc.gpsimd.load_library(mlp)

            # ---- early constants (needed by dense + scatter) ----
            w1_t = cpool.tile([IN_CH, HID], bf)
            nc.sync.dma_start(out=w1_t[:], in_=w1_d[:])
            dest_t = cpool.tile([P, ncht], f32)
            nc.sync.dma_start(out=dest_t[:], in_=dest_d[:])
            xfull = cpool.tile([P, LOCN], bf)
            nc.sync.dma_start(out=xfull[:], in_=xT_d[:])
            idx_t = cpool.tile([P, ncht * 8], i16)
            nc.sync.dma_start(out=idx_t[:], in_=idx_d[:])

            iota_i = cpool.tile([P, P], mybir.dt.int32)
            nc.gpsimd.iota(iota_i[:], pattern=[[1, P]], base=0,
                           channel_multiplier=0)
            iota_b = cpool.tile([P, P], bf)
            nc.vector.tensor_copy(out=iota_b[:], in_=iota_i[:])
            ident_f = cpool.tile([P, P], f32)
            make_identity(nc, ident_f[:])

            # late constants (not needed until post1 of half A)
            dinvimg = cpool.tile([P, LT * HID], bf)
            d2img = cpool.tile([P, LT * HID2], bf)
            w2_t = cpool.tile([HID, HID2], f32)
            t1_t = cpool.tile([P, HID], f32)
            t2_t = cpool.tile([P, HID2], f32)
            fcw_t = cpool.tile([P, HID2], f32)

            u1bf = upool.tile([P, LT, HID], bf, tag="u1b")
            s2bf = upool.tile([P, LT, HID2], bf, tag="s2")
            agg1 = upool.tile([P, LT, HID], bf, tag="agg1")
            agg2 = upool.tile([P, LT, HID2], f32, tag="agg2")
            h1 = upool.tile([P, LT, HID], f32, tag="h1")
            h2 = upool.tile([P, LT, HID2], f32, tag="h2")
            s2raw = upool.tile([P, LT * HID2], f32, tag="s2raw")
            y_sb = upool.tile([P, LT], f32, tag="y")

            tab1 = dpool.tile([LOCN, TBW], bf)
            tab2 = dpool.tile([LOCN, TBW], bf)
            part1a = dpool.tile([P, T_A * HID], bf)
            part1b = dpool.tile([P, (T_ALL - T_A) * HID], bf)
            part2a = dpool.tile([P, T_A * HID2], f32)
            part2b = dpool.tile([P, (T_ALL - T_A) * HID2], f32)
            rs1a = dpool.tile([SLAB, T_A * HID], bf)
            rs1b = dpool.tile([SLAB, (T_ALL - T_A) * HID], bf)
            rs2a = dpool.tile([SLAB, T_A * HID2], f32)
            rs2b = dpool.tile([SLAB, (T_ALL - T_A) * HID2], f32)

            # ---- L1 dense: u1 = (dinv*x) @ W1' ----
            for g in range(0, LT, 8):
                gl = min(8, LT - g)
                pm = ps2p.tile([P, GRP2, HID2], f32, space="PSUM", tag="ps2")
                pmv = pm[:].rearrange("p a w -> p (a w)")
                for j in range(gl):
                    nc.tensor.matmul(
                        out=pmv[:, j * HID:(j + 1) * HID],
                        lhsT=xfull[:, (g + j) * P:(g + j + 1) * P],
                        rhs=w1_t[:], start=True, stop=True)
                nc.scalar.activation(
                    out=u1bf[:, g:g + gl, :].rearrange("p a w -> p (a w)"),
                    in_=pmv[:, 0:gl * HID], func=Act.Copy)
                nc.sync.dma_start(
                    out=bassm.AP(tensor=tab1[:].tensor, offset=g * P * TBW,
                                 ap=[[TBW, P], [TBW * P, gl], [1, HID]]),
                    in_=u1bf[:, g:g + gl, :])

            selpool = tc.alloc_tile_pool(name="sel", bufs=SELB)

            # ---- scatter a range of dest tiles ----
            def scatter(tab, width, partial, grp, acc_dt, t0, t1):
                tab_ap = bassm.AP(tensor=tab[:].tensor, offset=0,
                                  ap=[[TBW, LOCN], [1, width]])
                acc = None
                accv = None
                call_start = 0
                call_len = 0
                ga = None
                for b in range(t0, t1):
                    gi, sl = divmod(b - t0, grp)
                    gl = min(grp, t1 - t0 - gi * grp)
                    if sl == 0:
                        acc = pacc.tile([P, grp, width], f32, space="PSUM",
                                        tag="acc")
                        accv = acc[:].rearrange("p a w -> p (a w)")
                    for j in range(int(ncb[b])):
                        k = int(bin_chunk_off[b]) + j
                        if k >= call_start + call_len:
                            call_start = k
                            call_len = min(NG, int(bin_chunk_off[t1]) - k)
                            ga = gapool.tile([P, call_len, width], bf, tag="ga")
                            _dma_gather_raw(
                                nc.gpsimd, nc, ga[:], tab_ap,
                                idx_t[:, call_start * 8:
                                      (call_start + call_len) * 8],
                                call_len * P, width, TBW)
                        sel = selpool.tile([P, P], bf, tag="sel")
                        nc.vector.tensor_scalar(
                            out=sel[:], in0=iota_b[:],
                            scalar1=dest_t[:, k:k + 1], scalar2=None,
                            op0=Alu.is_equal)
                        nc.tensor.matmul(
                            out=accv[:, sl * width:(sl + 1) * width],
                            lhsT=sel[:], rhs=ga[:, k - call_start, :],
                            start=(j == 0), stop=(j == int(ncb[b]) - 1))
                    if sl == gl - 1:
                        half = gi % 2
                        if half == 0:
                            asb = asbpool.tile([P, 2 * grp * width], acc_dt,
                                               tag="asb")
                        nc.scalar.activation(
                            out=asb[:, half * grp * width:
                                    half * grp * width + gl * width],
                            in_=accv[:, 0:gl * width], func=Act.Copy)
                        last = b == t1 - 1
                        if half == 1 or last:
                            wo = (gi - half) * grp * width
                            wn = (half * grp + gl) * width
                            nc.sync.dma_start(
                                out=partial[:, wo:wo + wn],
                                in_=asb[:, 0:wn])

            def reduce_scatter(partial, rsbuf):
                nc.gpsimd.collective_compute(
                    "ReduceScatter", mybir.AluOpType.add,
                    replica_groups=[list(range(NCORES))],
                    ins=[partial[:]],
                    outs=[rsbuf[:]],
                )

            def regroup(rsbuf, aggt, width, t0, t1):
                c0, c1 = t0 // 8, t1 // 8
                ctn = c1 - c0
                for a in range(SLAB):
                    eng = nc.sync if a % 2 == 0 else nc.gpsimd
                    eng.dma_start(
                        out=aggt[a * 8:(a + 1) * 8, c0:c1, :],
                        in_=bassm.AP(tensor=rsbuf[:].tensor,
                                     offset=a * (t1 - t0) * width,
                                     ap=[[width, 8], [8 * width, ctn],
                                         [1, width]]))

            # post1 for a ct range: h1 = relu((agg+u1)*dinv + T1);
            # s2 = dinv*(h1 @ W2'); write tab2 rows
            def post1(c0, c1):
                # pipelined per 4-ct groups: DVE -> Act relu -> PE transpose
                # -> Act drain -> PE W2 -> Act drain -> DVE s2 -> tab2 write
                for g in range(c0, c1, 4):
                    gl = min(4, c1 - g)
                    sl3 = (slice(None), slice(g, g + gl), slice(None))
                    fl = (slice(None), slice(g * HID, (g + gl) * HID))
                    nc.vector.tensor_tensor(out=h1[sl3], in0=agg1[sl3],
                                            in1=u1bf[sl3], op=Alu.add)
                    nc.vector.tensor_tensor(
                        out=h1[sl3].rearrange("p c w -> p (c w)"),
                        in0=h1[sl3].rearrange("p c w -> p (c w)"),
                        in1=dinvimg[fl], op=Alu.mult)
                    nc.vector.tensor_tensor(
                        out=h1[sl3], in0=h1[sl3],
                        in1=t1_t[:, None, :].to_broadcast([P, gl, HID]),
                        op=Alu.add)
                    nc.scalar.activation(
                        out=h1[sl3].rearrange("p c w -> p (c w)"),
                        in_=h1[sl3].rearrange("p c w -> p (c w)"),
                        func=Act.Relu)
                    tp = ptpp.tile([HID, 4, P], f32, space="PSUM", tag="tp")
                    for j in range(gl):
                        nc.tensor.transpose(out=tp[:, j, :],
                                            in_=h1[:, g + j, :],
                                            identity=ident_f[:])
                    hts = htspool.tile([HID, 4 * P], f32, tag="hts")
                    nc.scalar.activation(
                        out=hts[:, 0:gl * P],
                        in_=tp[:].rearrange("p a w -> p (a w)")[:, 0:gl * P],
                        func=Act.Copy)
                    pm2 = ps2p.tile([P, GRP2, HID2], f32, space="PSUM",
                                    tag="ps2")
                    pm2v = pm2[:].rearrange("p a w -> p (a w)")
                    for j in range(gl):
                        nc.tensor.matmul(
                            out=pm2v[:, j * HID2:(j + 1) * HID2],
                            lhsT=hts[:, j * P:(j + 1) * P], rhs=w2_t[:],
                            start=True, stop=True)
                    fl2 = (slice(None), slice(g * HID2, (g + gl) * HID2))
                    nc.scalar.activation(out=s2raw[:, g * HID2:
                                                   (g + gl) * HID2],
                                         in_=pm2v[:, 0:gl * HID2],
                                         func=Act.Copy)
                    nc.vector.tensor_tensor(
                        out=s2raw[fl2], in0=s2raw[fl2], in1=d2img[fl2],
                        op=Alu.mult)
                    nc.scalar.activation(
                        out=s2bf[sl3].rearrange("p c w -> p (c w)"),
                        in_=s2raw[fl2], func=Act.Copy)
                    nc.sync.dma_start(
                        out=bassm.AP(tensor=tab2[:].tensor,
                                     offset=g * P * TBW,
                                     ap=[[TBW, P], [TBW * P, gl], [1, HID2]]),
                        in_=s2bf[sl3])

            def post2(c0, c1):
                sl3 = (slice(None), slice(c0, c1), slice(None))
                fl2 = (slice(None), slice(c0 * HID2, c1 * HID2))
                nc.vector.tensor_tensor(
                    out=h2[sl3].rearrange("p c w -> p (c w)"),
                    in0=agg2[sl3].rearrange("p c w -> p (c w)"),
                    in1=s2raw[fl2], op=Alu.add)
                nc.vector.tensor_tensor(
                    out=h2[sl3].rearrange("p c w -> p (c w)"),
                    in0=h2[sl3].rearrange("p c w -> p (c w)"),
                    in1=d2img[fl2], op=Alu.mult)
                nc.vector.tensor_tensor(
                    out=h2[sl3], in0=h2[sl3],
                    in1=t2_t[:, None, :].to_broadcast([P, c1 - c0, HID2]),
                    op=Alu.add)
                nc.scalar.activation(
                    out=h2[sl3].rearrange("p c w -> p (c w)"),
                    in_=h2[sl3].rearrange("p c w -> p (c w)"), func=Act.Relu)
                nc.vector.tensor_tensor(
                    out=h2[sl3], in0=h2[sl3],
                    in1=fcw_t[:, None, :].to_broadcast([P, c1 - c0, HID2]),
                    op=Alu.mult)
                for c in range(c0, c1):
                    nc.vector.reduce_sum(out=y_sb[:, c:c + 1],
                                         in_=h2[:, c, :],
                                         axis=mybir.AxisListType.X)

            # ---- layer 1, half-pipelined ----
            scatter(tab1, HID, part1a, GRP1, bf, 0, T_A)
            reduce_scatter(part1a, rs1a)
            # late consts load during half B scatter
            nc.sync.dma_start(out=dinvimg[:], in_=dinvimg_d[:])
            nc.sync.dma_start(out=d2img[:], in_=d2img_d[:])
            nc.sync.dma_start(out=w2_t[:], in_=w2_d[:])
            nc.sync.dma_start(out=t1_t[:], in_=t1_d[:])
            nc.sync.dma_start(out=t2_t[:], in_=t2_d[:])
            nc.sync.dma_start(out=fcw_t[:], in_=fcw_d[:])
            scatter(tab1, HID, part1b, GRP1, bf, T_A, T_ALL)
            reduce_scatter(part1b, rs1b)
            regroup(rs1a, agg1, HID, 0, T_A)
            post1(0, CT_A)
            regroup(rs1b, agg1, HID, T_A, T_ALL)
            post1(CT_A, LT)

            # ---- layer 2, half-pipelined ----
            scatter(tab2, HID2, part2a, GRP2, f32, 0, T_A)
            reduce_scatter(part2a, rs2a)
            scatter(tab2, HID2, part2b, GRP2, f32, T_A, T_ALL)
            reduce_scatter(part2b, rs2b)
            regroup(rs2a, agg2, HID2, 0, T_A)
            post2(0, CT_A)
            regroup(rs2b, agg2, HID2, T_A, T_ALL)
            post2(CT_A, LT)
            nc.sync.dma_start(out=y_d[:], in_=y_sb[:])

    nc.compile()
    return nc


# ----------------------------------------------------------------------
# entry point
# ----------------------------------------------------------------------
def prepare(inputs):
    inputs = {k: np.asarray(v) for k, v in inputs.items()}
    cores, consts = host_prep(**inputs)
    nc = build_bass(consts["ncb"], consts["ncht"])

    in_maps = []
    for c in range(NCORES):
        in_maps.append({
            "xT": cores[c]["xT"],
            "idx": cores[c]["idx"],
            "dest": cores[c]["dest"],
            "dinvimg": cores[c]["dinvimg"],
            "d2img": cores[c]["d2img"],
            "w1": consts["w1"],
            "w2": consts["w2"],
            "t1": consts["t1"],
            "t2": consts["t2"],
            "fcw": consts["fcw"],
        })
    return nc, in_maps, consts


def execute(nc, in_maps):
    from concourse.bass_utils import run_bass_kernel_spmd
    return run_bass_kernel_spmd(nc, in_maps, core_ids=list(range(NCORES)))


def unshard(res, consts):
    y = np.zeros((N_NODES, 1), np.float32)
    owner, part, ct = consts["owner"], consts["part"], consts["ct"]
    fcb = consts["fcb"]
    pc = np.stack([np.asarray(res.results[c]["y"], np.float32)
                   for c in range(NCORES)])
    y[:, 0] = pc[owner[:N_NODES], part[:N_NODES], ct[:N_NODES]] + fcb
    return y


def kernel(**inputs):
    nc, in_maps, consts = prepare(inputs)
    res = execute(nc, in_maps)
    return unshard(res, consts)


# revision 43
# speedup vs baseline: 1.5460x; 1.0258x over previous
"""Distributed 2-layer GCN (BangaloreGCN) on 8 Trainium2 NeuronCores.

Source-partitioned strategy (node/graph parallel per the sharding hint,
with the cross-core reduction done by ReduceScatter instead of
AllGather):

  * Nodes are packed into 424 global dest tiles x 128 lanes; lanes
    [16c, 16c+16) of every tile belong to core c, so each core owns
    6784 node slots.  A color-aware greedy pack balances, for every
    (src core, dest tile) pair, the number of incoming edges to
    <= 256, so every dest tile needs exactly ceil/128 = NCH_b chunks
    (identical across cores -> one static SPMD program).
  * GCN algebra: per layer the table s = dinv*h is computed locally
    (8x less dense work), each core gathers s[src] for the edges whose
    SOURCE it owns from its local table, and scatter-adds them into a
    full-size partial accumulator [128 lanes, 424*64] with one-hot
    selection matmuls in PSUM.  A bf16 ReduceScatter then hands every
    core the complete sums for its own 16-lane slab.
  * One-hot masks are built per chunk with tensor_scalar(is_equal)
    (iota vs the chunk's dest-lane column), which runs in the DVE 4x
    perf mode.
  * Layer 2 applies W2 *before* aggregation (A@(hW2) == (A@h)W2), so
    its table rows are 32 wide: half the gather bytes and half the
    collective payload of layer 1.
"""

import sys

sys.path.insert(0, "/opt/trn_rl_repo")

import heapq

import ml_dtypes
import numpy as np

BF16 = ml_dtypes.bfloat16

# ---- problem constants (hardcoded per contest contract) ----
N_NODES = 50000
IN_CH = 128
HID = 64
HID2 = 32
BN_EPS = 1e-5

NCORES = 8
P = 128
T_ALL = 424                # global dest tiles
SLAB = 16                  # lanes per core in each tile
LOCN = T_ALL * SLAB        # local node slots per core (6784)
LT = LOCN // P             # local col-tiles (53)
NG = 64                    # chunks per dma_gather call
T_A = 256                  # tiles in pipeline half A (ct 0..31); rest in B
CT_A = T_A // 8
PAD_LANE = 200.0
TBW = 128                  # table row stride in bf16 elems (256B)
GRP1 = 8                   # dest bins per PSUM bank, layer 1 (8*64 = 512 f32)
GRP2 = 16                  # dest bins per PSUM bank, layer 2 (16*32 = 512 f32)


# ----------------------------------------------------------------------
# host-side preparation
# ----------------------------------------------------------------------
def _pack_nodes(row, col, n):
    """Assign every node an owner slab and a global dest tile + lane.

    Returns (lane, tile) per node: owner core = lane // 16."""
    deg_in = np.bincount(col, minlength=n)
    outdeg = np.bincount(row, minlength=n)

    # owner slabs: balance out-degree (work per core), capacity LOCN
    order = np.argsort(-outdeg, kind="stable")
    heap = [(0, c) for c in range(NCORES)]
    heapq.heapify(heap)
    cnt = np.zeros(NCORES, np.int64)
    owner = np.empty(n, np.int8)
    for v in order:
        load, c = heapq.heappop(heap)
        owner[v] = c
        cnt[c] += 1
        if cnt[c] < LOCN:
            heapq.heappush(heap, (load + int(outdeg[v]), c))

    # per-node in-edge color vector (color = owner of the source node)
    cc = np.zeros((n, NCORES), np.int32)
    np.add.at(cc, (col, owner[row].astype(np.int64)), 1)

    # color-aware greedy tile packing: keep max_c E_cb under the per-tile
    # chunk budget (light tiles aim for 1 chunk, the rest for 2)
    loads = np.zeros((T_ALL, NCORES), np.int64)
    cap = np.full((T_ALL, NCORES), SLAB, np.int16)
    budget = np.full(T_ALL, 2 * P, np.float64)
    budget[T_ALL - NLIGHT:] = P
    tile_of = np.empty(n, np.int32)
    lane_of = np.empty(n, np.int32)
    BIG = 1 << 40
    for v in np.argsort(-deg_in, kind="stable"):
        c = int(owner[v])
        score = (loads + cc[v][None, :]).max(axis=1) / budget
        score[cap[:, c] <= 0] = BIG
        t = int(np.argmin(score))
        tile_of[v] = t
        lane_of[v] = SLAB * c + (SLAB - cap[t, c])
        loads[t] += cc[v]
        cap[t, c] -= 1
    return lane_of, tile_of, loads


def _wrap_idx(arr):
    ni = arr.shape[0]
    blk = arr.reshape(ni // 16, 16).T.astype(np.int16)
    return np.tile(blk, (8, 1))


def host_prep(x, edge_index, W1, b1, W2, b2, fcW, fcb,
              g1, be1, rm1, rv1, g2, be2, rm2, rv2):
    n = x.shape[0]
    row = np.asarray(edge_index[0], np.int64)
    col = np.asarray(edge_index[1], np.int64)

    deg = np.bincount(col, minlength=n).astype(np.float32) + 1.0
    dinv = (1.0 / np.sqrt(deg)).astype(np.float32)

    lane, tile = _pack_nodes(row, col, n)[:2]
    owner = lane // SLAB
    a = lane % SLAB
    part = a * 8 + (tile % 8)          # SBUF partition in compute layout
    ct = tile // 8                     # SBUF col-tile in compute layout
    q = ct * P + part                  # compute index within core
    # gather tables are stored in compute order: table row == q

    # ---- per-(core, bin) chunk schedule, identical across cores ----
    e_core = owner[row]
    e_src = q[row].astype(np.int16)
    e_lane = lane[col].astype(np.float32)
    e_bin = tile[col]
    cnt_cb = np.zeros((NCORES, T_ALL), np.int64)
    np.add.at(cnt_cb, (e_core, e_bin), 1)
    ncb = np.maximum(1, -(-cnt_cb.max(axis=0) // P)).astype(np.int64)
    ncht = int(ncb.sum())
    bin_chunk_off = np.concatenate([[0], np.cumsum(ncb)])  # chunk offsets
    slot_off = bin_chunk_off * P

    # ---- per-core edge streams ----
    cores = []
    for c in range(NCORES):
        m = e_core == c
        sbin = e_bin[m]
        order = np.argsort(sbin, kind="stable")
        sbin = sbin[order]
        ssrc = e_src[m][order]
        slane = e_lane[m][order]
        starts = np.searchsorted(sbin, np.arange(T_ALL))
        rank = np.arange(len(sbin)) - starts[sbin]
        pos = slot_off[sbin] + rank
        idx_stream = np.zeros(ncht * P, np.int16)
        lane_stream = np.full(ncht * P, PAD_LANE, np.float32)
        idx_stream[pos] = ssrc
        lane_stream[pos] = slane

        calls = []
        half_edge = int(bin_chunk_off[T_A])
        for lo, hi in ((0, half_edge), (half_edge, ncht)):
            k = lo
            while k < hi:
                L = min(NG, hi - k)
                calls.append(_wrap_idx(idx_stream[k * P:(k + L) * P]))
                k += L
        idx_img = np.hstack(calls)
        dest_img = lane_stream.reshape(ncht, P).T.copy()

        nodes_c = np.where(owner == c)[0]
        xs = np.zeros((LOCN, IN_CH), np.float32)
        xs[q[nodes_c]] = x[nodes_c] * dinv[nodes_c, None]
        dv = np.zeros(LOCN, np.float32)
        dv[q[nodes_c]] = dinv[nodes_c]
        dvpt = dv.reshape(LT, P).T                      # [128, LT]
        cores.append(dict(
            idx=idx_img, dest=dest_img,
            xT=np.ascontiguousarray(xs.T).astype(BF16),
            dinvimg=np.ascontiguousarray(np.repeat(dvpt, HID, axis=1)).astype(BF16),
            d2img=np.ascontiguousarray(np.repeat(dvpt, HID2, axis=1)).astype(BF16),
        ))

    S1c = (g1 / np.sqrt(rv1 + BN_EPS)).astype(np.float32)
    T1 = ((b1 - rm1) * S1c + be1).astype(np.float32)
    S2c = (g2 / np.sqrt(rv2 + BN_EPS)).astype(np.float32)
    T2 = ((b2 - rm2) * S2c + be2).astype(np.float32)
    consts = dict(
        w1=(W1 * S1c[None, :]).astype(BF16),
        w2=(W2 * S2c[None, :]).astype(np.float32),
        t1=np.tile(T1[None, :], (P, 1)).astype(np.float32),
        t2=np.tile(T2[None, :], (P, 1)).astype(np.float32),
        fcw=np.tile(np.asarray(fcW, np.float32).reshape(1, -1), (P, 1)).astype(np.float32),
        fcb=float(np.asarray(fcb).reshape(-1)[0]),
        ncb=ncb, ncht=ncht,
        owner=owner, part=part, ct=ct,
    )
    return cores, consts


# ----------------------------------------------------------------------
# device program
# ----------------------------------------------------------------------
def _dma_gather_raw(gp, bassmod, out_ap, in_ap, idxs_ap, num_idxs, elem_size,
                    elem_step, single_packet=False, queue_num=0):
    """bass.dma_gather allowing elem_size_bytes below 256B (row stride must
    still be a multiple of 256B)."""
    import concourse.mybir as mybir
    from concourse import ap_utils
    from concourse.bass import MemorySpace, exact_div, round_up_to_multiple

    assert idxs_ap.dtype == mybir.dt.int16
    assert in_ap.dtype == out_ap.dtype
    assert in_ap.space == MemorySpace.DRAM
    assert idxs_ap.space == MemorySpace.SBUF and out_ap.space == MemorySpace.SBUF
    assert ap_utils.ap_is_contiguous(out_ap.ap[1:])
    assert ap_utils.ap_is_contiguous(idxs_ap.ap[1:])
    assert in_ap.ap[-1][1] == out_ap.ap[-1][1] == elem_size
    assert out_ap.ap[0][1] * out_ap.ap[1][1] == round_up_to_multiple(num_idxs, 128)
    assert in_ap.ap[0][0] == elem_step
    stride_bytes_256 = exact_div(elem_step * mybir.dt.size(in_ap.dtype), 256)
    assert stride_bytes_256 < 256
    return gp.add_instruction(
        mybir.InstDMAGatherAnt(
            name=bassmod.get_next_instruction_name(),
            ins=[*gp.lower_ap_dma(in_ap, for_custom_bir_dma=True),
                 gp.lower_ap(idxs_ap),
                 gp.lower_val_access(gp.to_reg(num_idxs))],
            outs=[gp.lower_ap(out_ap)],
            transpose=False,
            num_idxs=num_idxs,
            elem_size=elem_size,
            stride_bytes_256=stride_bytes_256,
            gen_mode=0,
            single_packet=single_packet,
            queue_num=queue_num,
            sbuf_tokens_per_rank=0,
            sbuf_free_dim_per_rank=0,
            sbuf_free_dim_pad_per_rank=0,
            sbuf_byte_offset=0,
        ))


def build_bass(ncb, ncht):
    import concourse.bacc as bacc
    import concourse.bass as bassm
    import concourse.mybir as mybir
    import concourse.tile as tile
    from concourse.library_config import mlp
    from concourse.masks import make_identity

    f32 = mybir.dt.float32
    bf = mybir.dt.bfloat16
    i16 = mybir.dt.int16
    Act = mybir.ActivationFunctionType
    Alu = mybir.AluOpType

    bin_chunk_off = np.concatenate([[0], np.cumsum(ncb)])
    HALVES = [(0, T_A), (T_A, T_ALL)]

    nc = bacc.Bacc("TRN2", target_bir_lowering=False)
    xT_d = nc.dram_tensor("xT", [P, LOCN], bf, kind="ExternalInput")
    idx_d = nc.dram_tensor("idx", [P, ncht * 8], i16, kind="ExternalInput")
    dest_d = nc.dram_tensor("dest", [P, ncht], f32, kind="ExternalInput")
    dinvimg_d = nc.dram_tensor("dinvimg", [P, LT * HID], bf, kind="ExternalInput")
    d2img_d = nc.dram_tensor("d2img", [P, LT * HID2], bf, kind="ExternalInput")
    w1_d = nc.dram_tensor("w1", [IN_CH, HID], bf, kind="ExternalInput")
    w2_d = nc.dram_tensor("w2", [HID, HID2], f32, kind="ExternalInput")
    t1_d = nc.dram_tensor("t1", [P, HID], f32, kind="ExternalInput")
    t2_d = nc.dram_tensor("t2", [P, HID2], f32, kind="ExternalInput")
    fcw_d = nc.dram_tensor("fcw", [P, HID2], f32, kind="ExternalInput")
    y_d = nc.dram_tensor("y", [P, LT], f32, kind="ExternalOutput")

    with tile.TileContext(nc) as tc:
        with (
            tc.tile_pool(name="const", bufs=1) as cpool,
            tc.tile_pool(name="upart", bufs=1) as upool,
            tc.tile_pool(name="ga", bufs=3) as gapool,
            tc.tile_pool(name="sel", bufs=24) as selpool,
            tc.tile_pool(name="asb", bufs=3) as asbpool,
            tc.tile_pool(name="hts", bufs=2) as htspool,
            tc.tile_pool(name="pacc", bufs=3, space="PSUM") as pacc,
            tc.tile_pool(name="ps2", bufs=2, space="PSUM") as ps2p,
            tc.tile_pool(name="ptp", bufs=2, space="PSUM") as ptpp,
            tc.tile_pool(name="dram", bufs=1, space="DRAM") as dpool,
        ):
            nc.gpsimd.load_library(mlp)

            # ---- early constants (needed by dense + scatter) ----
            w1_t = cpool.tile([IN_CH, HID], bf)
            nc.sync.dma_start(out=w1_t[:], in_=w1_d[:])
            dest_t = cpool.tile([P, ncht], f32)
            nc.sync.dma_start(out=dest_t[:], in_=dest_d[:])
            xfull = cpool.tile([P, LOCN], bf)
            nc.sync.dma_start(out=xfull[:], in_=xT_d[:])
            idx_t = cpool.tile([P, ncht * 8], i16)
            nc.sync.dma_start(out=idx_t[:], in_=idx_d[:])

            iota_i = cpool.tile([P, P], mybir.dt.int32)
            nc.gpsimd.iota(iota_i[:], pattern=[[1, P]], base=0,
                           channel_multiplier=0)
            iota_b = cpool.tile([P, P], bf)
            nc.vector.tensor_copy(out=iota_b[:], in_=iota_i[:])
            ident_f = cpool.tile([P, P], f32)
            make_identity(nc, ident_f[:])

            # late constants (not needed until post1 of half A)
            dinvimg = cpool.tile([P, LT * HID], bf)
            d2img = cpool.tile([P, LT * HID2], bf)
            w2_t = cpool.tile([HID, HID2], f32)
            t1_t = cpool.tile([P, HID], f32)
            t2_t = cpool.tile([P, HID2], f32)
            fcw_t = cpool.tile([P, HID2], f32)

            u1bf = upool.tile([P, LT, HID], bf, tag="u1b")
            s2bf = upool.tile([P, LT, HID2], bf, tag="s2")
            agg1 = upool.tile([P, LT, HID], bf, tag="agg1")
            agg2 = upool.tile([P, LT, HID2], bf, tag="agg2")
            h1 = upool.tile([P, LT, HID], f32, tag="h1")
            h2 = upool.tile([P, LT, HID2], f32, tag="h2")
            s2raw = upool.tile([P, LT * HID2], f32, tag="s2raw")
            y_sb = upool.tile([P, LT], f32, tag="y")

            tab1 = dpool.tile([LOCN, TBW], bf)
            tab2 = dpool.tile([LOCN, TBW], bf)
            part1a = dpool.tile([P, T_A * HID], bf)
            part1b = dpool.tile([P, (T_ALL - T_A) * HID], bf)
            part2a = dpool.tile([P, T_A * HID2], f32)
            part2b = dpool.tile([P, (T_ALL - T_A) * HID2], f32)
            rs1a = dpool.tile([SLAB, T_A * HID], bf)
            rs1b = dpool.tile([SLAB, (T_ALL - T_A) * HID], bf)
            rs2a = dpool.tile([SLAB, T_A * HID2], f32)
            rs2b = dpool.tile([SLAB, (T_ALL - T_A) * HID2], f32)

            # ---- L1 dense: u1 = (dinv*x) @ W1' ----
            for g in range(0, LT, 8):
                gl = min(8, LT - g)
                pm = ps2p.tile([P, GRP2, HID2], f32, space="PSUM", tag="ps2")
                pmv = pm[:].rearrange("p a w -> p (a w)")
                for j in range(gl):
                    nc.tensor.matmul(
                        out=pmv[:, j * HID:(j + 1) * HID],
                        lhsT=xfull[:, (g + j) * P:(g + j + 1) * P],
                        rhs=w1_t[:], start=True, stop=True)
                nc.scalar.activation(
                    out=u1bf[:, g:g + gl, :].rearrange("p a w -> p (a w)"),
                    in_=pmv[:, 0:gl * HID], func=Act.Copy)
                nc.sync.dma_start(
                    out=bassm.AP(tensor=tab1[:].tensor, offset=g * P * TBW,
                                 ap=[[TBW, P], [TBW * P, gl], [1, HID]]),
                    in_=u1bf[:, g:g + gl, :])

            selpool = tc.alloc_tile_pool(name="sel", bufs=SELB)

            # ---- scatter a range of dest tiles ----
            def scatter(tab, width, partial, grp, acc_dt, t0, t1):
                tab_ap = bassm.AP(tensor=tab[:].tensor, offset=0,
                                  ap=[[TBW, LOCN], [1, width]])
                acc = None
                accv = None
                call_start = 0
                call_len = 0
                ga = None
                for b in range(t0, t1):
                    gi, sl = divmod(b - t0, grp)
                    gl = min(grp, t1 - t0 - gi * grp)
                    if sl == 0:
                        acc = pacc.tile([P, grp, width], f32, space="PSUM",
                                        tag="acc")
                        accv = acc[:].rearrange("p a w -> p (a w)")
                    for j in range(int(ncb[b])):
                        k = int(bin_chunk_off[b]) + j
                        if k >= call_start + call_len:
                            call_start = k
                            call_len = min(NG, int(bin_chunk_off[t1]) - k)
                            ga = gapool.tile([P, call_len, width], bf, tag="ga")
                            _dma_gather_raw(
                                nc.gpsimd, nc, ga[:], tab_ap,
                                idx_t[:, call_start * 8:
                                      (call_start + call_len) * 8],
                                call_len * P, width, TBW)
                        sel = selpool.tile([P, P], bf, tag="sel")
                        nc.vector.tensor_scalar(
                            out=sel[:], in0=iota_b[:],
                            scalar1=dest_t[:, k:k + 1], scalar2=None,
                            op0=Alu.is_equal)
                        nc.tensor.matmul(
                            out=accv[:, sl * width:(sl + 1) * width],
                            lhsT=sel[:], rhs=ga[:, k - call_start, :],
                            start=(j == 0), stop=(j == int(ncb[b]) - 1))
                    if sl == gl - 1:
                        half = gi % 2
                        if half == 0:
                            asb = asbpool.tile([P, 2 * grp * width], acc_dt,
                                               tag="asb")
                        nc.scalar.activation(
                            out=asb[:, half * grp * width:
                                    half * grp * width + gl * width],
                            in_=accv[:, 0:gl * width], func=Act.Copy)
                        last = b == t1 - 1
                        if half == 1 or last:
                            wo = (gi - half) * grp * width
                            wn = (half * grp + gl) * width
                            nc.sync.dma_start(
                                out=partial[:, wo:wo + wn],
                                in_=asb[:, 0:wn])

            def reduce_scatter(partial, rsbuf):
                nc.gpsimd.collective_compute(
                    "ReduceScatter", mybir.AluOpType.add,
                    replica_groups=[list(range(NCORES))],
                    ins=[partial[:]],
                    outs=[rsbuf[:]],
                )

            def regroup(rsbuf, aggt, width, t0, t1):
                c0, c1 = t0 // 8, t1 // 8
                ctn = c1 - c0
                for a in range(SLAB):
                    eng = nc.sync if a % 2 == 0 else nc.gpsimd
                    eng.dma_start(
                        out=aggt[a * 8:(a + 1) * 8, c0:c1, :],
                        in_=bassm.AP(tensor=rsbuf[:].tensor,
                                     offset=a * (t1 - t0) * width,
                                     ap=[[width, 8], [8 * width, ctn],
                                         [1, width]]))

            # post1 for a ct range: h1 = relu((agg+u1)*dinv + T1);
            # s2 = dinv*(h1 @ W2'); write tab2 rows
            def post1(c0, c1):
                # pipelined per 4-ct groups: DVE -> Act relu -> PE transpose
                # -> Act drain -> PE W2 -> Act drain -> DVE s2 -> tab2 write
                for g in range(c0, c1, 4):
                    gl = min(4, c1 - g)
                    sl3 = (slice(None), slice(g, g + gl), slice(None))
                    fl = (slice(None), slice(g * HID, (g + gl) * HID))
                    nc.vector.tensor_tensor(out=h1[sl3], in0=agg1[sl3],
                                            in1=u1bf[sl3], op=Alu.add)
                    nc.vector.tensor_tensor(
                        out=h1[sl3].rearrange("p c w -> p (c w)"),
                        in0=h1[sl3].rearrange("p c w -> p (c w)"),
                        in1=dinvimg[fl], op=Alu.mult)
                    nc.vector.tensor_tensor(
                        out=h1[sl3], in0=h1[sl3],
                        in1=t1_t[:, None, :].to_broadcast([P, gl, HID]),
                        op=Alu.add)
                    nc.scalar.activation(
                        out=h1[sl3].rearrange("p c w -> p (c w)"),
                        in_=h1[sl3].rearrange("p c w -> p (c w)"),
                        func=Act.Relu)
                    tp = ptpp.tile([HID, 4, P], f32, space="PSUM", tag="tp")
                    for j in range(gl):
                        nc.tensor.transpose(out=tp[:, j, :],
                                            in_=h1[:, g + j, :],
                                            identity=ident_f[:])
                    hts = htspool.tile([HID, 4 * P], f32, tag="hts")
                    nc.scalar.activation(
                        out=hts[:, 0:gl * P],
                        in_=tp[:].rearrange("p a w -> p (a w)")[:, 0:gl * P],
                        func=Act.Copy)
                    pm2 = ps2p.tile([P, GRP2, HID2], f32, space="PSUM",
                                    tag="ps2")
                    pm2v = pm2[:].rearrange("p a w -> p (a w)")
                    for j in range(gl):
                        nc.tensor.matmul(
                            out=pm2v[:, j * HID2:(j + 1) * HID2],
                            lhsT=hts[:, j * P:(j + 1) * P], rhs=w2_t[:],
                            start=True, stop=True)
                    fl2 = (slice(None), slice(g * HID2, (g + gl) * HID2))
                    nc.scalar.activation(out=s2raw[:, g * HID2:
                                                   (g + gl) * HID2],
                                         in_=pm2v[:, 0:gl * HID2],
                                         func=Act.Copy)
                    nc.vector.tensor_tensor(
                        out=s2raw[fl2], in0=s2raw[fl2], in1=d2img[fl2],
                        op=Alu.mult)
                    nc.scalar.activation(
                        out=s2bf[sl3].rearrange("p c w -> p (c w)"),
                        in_=s2raw[fl2], func=Act.Copy)
                    nc.sync.dma_start(
                        out=bassm.AP(tensor=tab2[:].tensor,
                                     offset=g * P * TBW,
                                     ap=[[TBW, P], [TBW * P, gl], [1, HID2]]),
                        in_=s2bf[sl3])

            def post2(c0, c1):
                sl3 = (slice(None), slice(c0, c1), slice(None))
                fl2 = (slice(None), slice(c0 * HID2, c1 * HID2))
                nc.vector.tensor_tensor(
                    out=h2[sl3].rearrange("p c w -> p (c w)"),
                    in0=agg2[sl3].rearrange("p c w -> p (c w)"),
                    in1=s2raw[fl2], op=Alu.add)
                nc.vector.tensor_tensor(
                    out=h2[sl3].rearrange("p c w -> p (c w)"),
                    in0=h2[sl3].rearrange("p c w -> p (c w)"),
                    in1=d2img[fl2], op=Alu.mult)
                nc.vector.tensor_tensor(
                    out=h2[sl3], in0=h2[sl3],
                    in1=t2_t[:, None, :].to_broadcast([P, c1 - c0, HID2]),
                    op=Alu.add)
                nc.scalar.activation(
                    out=h2[sl3].rearrange("p c w -> p (c w)"),
                    in_=h2[sl3].rearrange("p c w -> p (c w)"), func=Act.Relu)
                nc.vector.tensor_tensor(
                    out=h2[sl3], in0=h2[sl3],
                    in1=fcw_t[:, None, :].to_broadcast([P, c1 - c0, HID2]),
                    op=Alu.mult)
                for c in range(c0, c1):
                    nc.vector.reduce_sum(out=y_sb[:, c:c + 1],
                                         in_=h2[:, c, :],
                                         axis=mybir.AxisListType.X)

            # ---- layer 1, half-pipelined ----
            scatter(tab1, HID, part1a, GRP1, bf, 0, T_A)
            reduce_scatter(part1a, rs1a)
            # late consts load during half B scatter
            nc.sync.dma_start(out=dinvimg[:], in_=dinvimg_d[:])
            nc.sync.dma_start(out=d2img[:], in_=d2img_d[:])
            nc.sync.dma_start(out=w2_t[:], in_=w2_d[:])
            nc.sync.dma_start(out=t1_t[:], in_=t1_d[:])
            nc.sync.dma_start(out=t2_t[:], in_=t2_d[:])
            nc.sync.dma_start(out=fcw_t[:], in_=fcw_d[:])
            scatter(tab1, HID, part1b, GRP1, bf, T_A, T_ALL)
            reduce_scatter(part1b, rs1b)
            regroup(rs1a, agg1, HID, 0, T_A)
            post1(0, CT_A)
            regroup(rs1b, agg1, HID, T_A, T_ALL)
            post1(CT_A, LT)

            # ---- layer 2, half-pipelined ----
            scatter(tab2, HID2, part2a, GRP2, f32, 0, T_A)
            reduce_scatter(part2a, rs2a)
            scatter(tab2, HID2, part2b, GRP2, f32, T_A, T_ALL)
            reduce_scatter(part2b, rs2b)
            regroup(rs2a, agg2, HID2, 0, T_A)
            post2(0, CT_A)
            regroup(rs2b, agg2, HID2, T_A, T_ALL)
            post2(CT_A, LT)
            nc.sync.dma_start(out=y_d[:], in_=y_sb[:])

    nc.compile()
    return nc


# ----------------------------------------------------------------------
# entry point
# ----------------------------------------------------------------------
def prepare(inputs):
    inputs = {k: np.asarray(v) for k, v in inputs.items()}
    cores, consts = host_prep(**inputs)
    nc = build_bass(consts["ncb"], consts["ncht"])

    in_maps = []
    for c in range(NCORES):
        in_maps.append({
            "xT": cores[c]["xT"],
            "idx": cores[c]["idx"],
            "dest": cores[c]["dest"],
            "dinvimg": cores[c]["dinvimg"],
            "d2img": cores[c]["d2img"],
            "w1": consts["w1"],
            "w2": consts["w2"],
            "t1": consts["t1"],
            "t2": consts["t2"],
            "fcw": consts["fcw"],
        })
    return nc, in_maps, consts


def execute(nc, in_maps):
    from concourse.bass_utils import run_bass_kernel_spmd
    return run_bass_kernel_spmd(nc, in_maps, core_ids=list(range(NCORES)))


def unshard(res, consts):
    y = np.zeros((N_NODES, 1), np.float32)
    owner, part, ct = consts["owner"], consts["part"], consts["ct"]
    fcb = consts["fcb"]
    pc = np.stack([np.asarray(res.results[c]["y"], np.float32)
                   for c in range(NCORES)])
    y[:, 0] = pc[owner[:N_NODES], part[:N_NODES], ct[:N_NODES]] + fcb
    return y


def kernel(**inputs):
    nc, in_maps, consts = prepare(inputs)
    res = execute(nc, in_maps)
    return unshard(res, consts)
